# revision 68
# baseline (speedup 1.0000x reference)
"""Trainium2 Bass kernel for BondingGraphGNN (gnn_message_passing), v3.

Model (see reference):
  h = relu(x @ W_emb)
  4x: m = h @ W_msg[i]; agg = scatter_add(m[src] -> dst); h = GRU(agg, h)
  h = relu(h); pooled = segment_mean(h, batch); out = softplus(relu(pooled@W1+b1)@W2+b2)

Distribution: even node sharding (6250 nodes/core, padded). Per step each core
computes messages for its nodes, AllGathers the message table to DRAM, and
aggregates its incoming edges locally with a gather + one-hot-matmul
segment-sum, then runs the GRU.

v3 performance structure (~2.3x over v2 in the cost model):
- fp8e4 message table in pair-rows ([2 nodes, 256B] per row, parity-pure edge
  tiles pick their half via the lhsT offset): halves AllGather bytes; the
  scatter-sum averages ~16 messages so fp8 noise washes out (rel err ~1e-3).
- DoubleRow fp8 matmuls: two 128-edge tiles (256-deep contraction) per PE
  instruction via a custom strided lhsT AP - halves PE sequencer work, which
  otherwise bounds the aggregation.
- One DVE is_equal per (group, segment) cell builds the whole one-hot St
  stack (consecutive slot columns, stride-0 broadcast APs).
- Per-step AllGather in 3 pieces [16,17,17] tiles, consumed pass-by-pass
  (pass 0 initializes aggT, later passes accumulate). The small piece 0
  means the next step's first AllGather only waits on 4 GRU chunks, and the
  last pass's gathers hide under its flight - the collective engine streams
  nearly back-to-back.
- Step-invariant gather indices/slots preloaded once; startup reordered so
  the embedding's xT DMA precedes bulk prefetches; shard writes split at
  chunk boundaries so each fires as messages complete; readout folded into
  the last step's chunk loop; pool reduction via AllGather + local sum
  (cheaper than AllReduce); a tiny warmup AllGather absorbs communicator
  bootstrap before the first real collective.

Readout: per-core partial pooling + AllGather + local sum + tiny MLP
replicated on every core (host takes core 0).
"""

import os
import numpy as np

# the trimmed axon package in some containers lacks the NTFF profile hook
# module; stub it so run_bass_kernel_spmd(trace=True) degrades gracefully.
import sys as _sys, types as _types
try:
    import antenv.axon_hooks  # noqa: F401
except Exception:
    _m = _types.ModuleType("antenv.axon_hooks")
    _m.get_axon_ntff_profile_hook = lambda: None
    _sys.modules["antenv.axon_hooks"] = _m

import ml_dtypes
import concourse.bacc as bacc
import concourse.bass as bass
import concourse.mybir as mybir
import concourse.tile as tile
from concourse.bass_utils import run_bass_kernel_spmd

F32 = mybir.dt.float32
BF16 = mybir.dt.bfloat16
F8 = mybir.dt.float8e4
I16 = mybir.dt.int16
AF = mybir.ActivationFunctionType

N_NODES = 50000
N_EDGES = 800000
FEAT = 90
H = 128
STEPS = 4
N_GRAPHS = 100
N_CORES = 8

NC_NODES = N_NODES // N_CORES               # 6250 real nodes per core
N_PAD = 6400
NT = N_PAD // 128                           # dst groups per core (50)
N_CHUNKS = (NT + 3) // 4                    # 13 chunks of <=4 groups
G_PADG = 112                                # padded global graph count
PAD_SLOT = 255.0                            # sentinel slot -> all-zero S row

_TAB_NAME = os.environ.get("K_TAB", "f8")   # f32 | bf16 | f8
ROW_N = 2 if _TAB_NAME == "f8" else 1       # table nodes per row (fp8 pairs)
ROW_W = ROW_N * 128                         # table row width, elements


def _mk_layout(seg_tiles):
    """Per-step AllGather piece layout. Small first piece: the next step's
    first AllGather only waits on a few GRU chunks; small last piece: little
    post-collective aggregation work."""
    assert sum(seg_tiles) == NT
    nodes = [t * 128 for t in seg_tiles]
    start = [sum(nodes[:q]) for q in range(len(seg_tiles))]
    rows = [N_CORES * n for n in nodes]
    assert max(rows) // ROW_N <= 32768
    return dict(tiles=seg_tiles, nodes=nodes, start=start, rows=rows,
                n_ag=len(seg_tiles))


def _parse_segs(env, default):
    v = os.environ.get(env, "")
    return [int(x) for x in v.split(",")] if v else default


# per-step piece layouts (a distinct last-step layout is supported via
# K_SEGS_LAST but the uniform split benches best)
LAYOUTS = [_mk_layout(_parse_segs("K_SEGS", [16, 17, 17])),
           _mk_layout(_parse_segs("K_SEGS_LAST", [16, 17, 17]))]
STEP_LAYOUT = [0, 0, 0, 1]
if LAYOUTS[1]["tiles"] == LAYOUTS[0]["tiles"]:
    LAYOUTS = LAYOUTS[:1]
    STEP_LAYOUT = [0, 0, 0, 0]

TAB_DT_F32 = _TAB_NAME == "f32"
LAST_RESULTS = {}   # stash for test.py (exec time etc)


def _chunk_groups(ch):
    return range(ch * 4, min((ch + 1) * 4, NT))


# ----------------------------------------------------------------------------
# host-side layout
# ----------------------------------------------------------------------------

def _edge_layout(lay, d_core, grp, slot, s_core, s_local):
    """Tile/slot/idx tables for one AllGather piece layout."""
    n_seg = lay["n_ag"]
    bounds = np.asarray(lay["start"][1:] + [N_PAD], np.int64)
    seg = np.searchsorted(bounds, s_local, side="right")
    seg_nodes = np.asarray(lay["nodes"], np.int64)[seg]
    seg_start = np.asarray(lay["start"], np.int64)[seg]
    loc = s_core * seg_nodes + (s_local - seg_start)
    if ROW_N == 2:
        # fp8 pair-row table: row r of a piece holds sources (2r, 2r+1);
        # each tile is parity-pure so the matmul lhsT picks one half.
        par = loc % 2
        idxval = loc // 2
    else:
        par = np.zeros_like(loc)
        idxval = loc

    # per (core, grp, seg, par) counts -> uniform budgets
    cnt = np.zeros((N_CORES, NT, n_seg, ROW_N), np.int64)
    np.add.at(cnt, (d_core, grp, seg, par), 1)
    budget = np.ceil(cnt.max(axis=0) / 128).astype(np.int64)
    # every group needs >=1 pass-0 tile so the copy flush writes aggT
    need = budget[:, 0, :].sum(axis=1) == 0
    budget[need, 0, 0] = 1

    # tile order: seg-major, then chunk, then group, then parity
    tb = np.zeros((NT, n_seg, ROW_N), np.int64)
    chunk_t0 = np.zeros((N_CHUNKS, n_seg), np.int64)
    chunk_nt = np.zeros((N_CHUNKS, n_seg), np.int64)
    t = 0
    for s in range(n_seg):
        for ch in range(N_CHUNKS):
            chunk_t0[ch, s] = t
            for g in _chunk_groups(ch):
                for p in range(ROW_N):
                    tb[g, s, p] = t
                    t += int(budget[g, s, p])
            chunk_nt[ch, s] = t - chunk_t0[ch, s]
    t_tot = t

    # edge placement (vectorized)
    order = np.lexsort((par, grp, seg, d_core))
    sc = d_core[order]
    ss = seg[order]
    sg = grp[order]
    sp = par[order]
    sidx = idxval[order]
    sslot = slot[order]
    rid = ((sc * n_seg + ss) * NT + sg) * ROW_N + sp
    run_first = np.r_[0, np.flatnonzero(np.diff(rid)) + 1]
    run_len = np.diff(np.r_[run_first, len(rid)])
    k = np.arange(len(rid)) - np.repeat(run_first, run_len)
    tt = tb[sg, ss, sp] + k // 128
    pp = k % 128
    ct0 = chunk_t0[sg // 4, ss]
    pos = (tt - ct0) * 128 + pp

    idx_arr = np.zeros((N_CORES, 16, t_tot * 8), np.int16)
    slot_arr = np.full((N_CORES, 128, t_tot), PAD_SLOT, np.float32)
    idx_arr[sc, pos % 16, ct0 * 8 + pos // 16] = sidx.astype(np.int16)
    slot_arr[sc, pp, tt] = sslot
    return dict(budget=budget, tb=tb, chunk_t0=chunk_t0, chunk_nt=chunk_nt,
                t_tot=t_tot, idx_arr=idx_arr, slot_arr=slot_arr)


def _preprocess(x, edge_index, batch):
    batch = np.asarray(batch, np.int64)
    src = np.asarray(edge_index[0], np.int64)
    dst = np.asarray(edge_index[1], np.int64)
    frac = float(os.environ.get("K_EDGE_FRAC", "1"))
    if frac < 1.0:  # timing experiments only - wrong results
        n = int(len(src) * frac)
        src, dst = src[:n], dst[:n]

    d_core = dst // NC_NODES
    d_local = dst - d_core * NC_NODES
    grp = d_local // 128
    slot = (d_local % 128).astype(np.float32)
    s_core = src // NC_NODES
    s_local = src - s_core * NC_NODES

    els = [_edge_layout(lay, d_core, grp, slot, s_core, s_local)
           for lay in LAYOUTS]

    # per-core node features (transposed, padded, bf16) and graph one-hots
    counts = np.bincount(batch, minlength=N_GRAPHS).astype(np.float32)
    invc = np.zeros((G_PADG, 1), np.float32)
    invc[:N_GRAPHS, 0] = 1.0 / np.maximum(counts, 1.0)
    x = np.asarray(x, np.float32)
    slot_dt = np.float32 if TAB_DT_F32 else ml_dtypes.bfloat16
    per_core = []
    for c in range(N_CORES):
        n0 = c * NC_NODES
        xT = np.zeros((FEAT, N_PAD), np.float32)
        xT[:, :NC_NODES] = x[n0:n0 + NC_NODES].T
        gmat = np.zeros((128, NT * G_PADG), np.float32)
        l = np.arange(NC_NODES)
        gmat[l % 128, (l // 128) * G_PADG + batch[n0:n0 + NC_NODES]] = 1.0
        pc = dict(
            xT=xT.astype(ml_dtypes.bfloat16),
            gmat=gmat.astype(ml_dtypes.bfloat16),
        )
        for li, el in enumerate(els):
            pc[f"idx{li}"] = np.tile(el["idx_arr"][c], (8, 1))
            pc[f"slot{li}"] = el["slot_arr"][c].astype(slot_dt)
        per_core.append(pc)

    meta = dict(els=els, invc=invc)
    return per_core, meta


# ----------------------------------------------------------------------------
# device program
# ----------------------------------------------------------------------------

def _build(meta):
    DBG_STEPS = int(os.environ.get("K_STEPS", STEPS))
    DBG_NO_AG = bool(int(os.environ.get("K_NO_AG", "0")))
    DBG_NO_AGG = bool(int(os.environ.get("K_NO_AGG", "0")))
    DBG_NO_GRU = bool(int(os.environ.get("K_NO_GRU", "0")))
    els = meta["els"]

    nc = bacc.Bacc("TRN2", target_bir_lowering=False, debug=False,
                   num_devices=N_CORES)

    d_xT = nc.dram_tensor("xT", [FEAT, N_PAD], BF16, kind="ExternalInput")
    SLOT_DT = F32 if TAB_DT_F32 else BF16
    d_idx = [nc.dram_tensor(f"idx{li}", [128, el["t_tot"] * 8], I16,
                            kind="ExternalInput")
             for li, el in enumerate(els)]
    d_slot = [nc.dram_tensor(f"slot{li}", [128, el["t_tot"]], SLOT_DT,
                             kind="ExternalInput")
              for li, el in enumerate(els)]
    d_gmat = nc.dram_tensor("gmat", [128, NT * G_PADG], BF16,
                            kind="ExternalInput")
    d_invc = nc.dram_tensor("invc", [G_PADG, 1], F32, kind="ExternalInput")
    d_iota = nc.dram_tensor("iota", [1, 128], F32, kind="ExternalInput")
    d_ident = nc.dram_tensor("ident", [128, 128], BF16, kind="ExternalInput")
    d_wemb = nc.dram_tensor("wemb", [FEAT, H], BF16, kind="ExternalInput")
    d_wmsg = nc.dram_tensor("wmsg", [STEPS, H, H], BF16, kind="ExternalInput")
    d_wih = nc.dram_tensor("wih", [H, 3 * H], BF16, kind="ExternalInput")
    d_whh = nc.dram_tensor("whh", [H, 3 * H], BF16, kind="ExternalInput")
    d_bihT = nc.dram_tensor("bihT", [H, 3], F32, kind="ExternalInput")
    d_bhhT = nc.dram_tensor("bhhT", [H, 3], F32, kind="ExternalInput")
    d_w1 = nc.dram_tensor("w1", [H, H], BF16, kind="ExternalInput")
    d_b1 = nc.dram_tensor("b1", [H, 1], F32, kind="ExternalInput")
    d_w2 = nc.dram_tensor("w2", [H, 1], BF16, kind="ExternalInput")
    d_b2 = nc.dram_tensor("b2", [1, 1], F32, kind="ExternalInput")
    d_out = nc.dram_tensor("out", [1, G_PADG], F32, kind="ExternalOutput")

    with tile.TileContext(nc) as tc:
        with (
            tc.tile_pool(name="persist", bufs=1) as P,
            tc.tile_pool(name="dram", bufs=1, space="DRAM") as DR,
            tc.tile_pool(name="epool",
                         bufs=int(os.environ.get("K_EP", "4"))) as EP,
            tc.tile_pool(name="spool",
                         bufs=int(os.environ.get("K_SPOOL", "8"))) as SP,
            tc.tile_pool(name="gpool", bufs=2) as GP,
            tc.tile_pool(name="ps_agg", space="PSUM",
                         bufs=int(os.environ.get("K_PA", "2"))) as PS_AGG,
            tc.tile_pool(name="ps_m", bufs=1, space="PSUM") as PS_M,
            tc.tile_pool(name="ps_gru", space="PSUM",
                         bufs=int(os.environ.get("K_PG", "4"))) as PS_GRU,
        ):
            # DRAM temps: per-step message shards and gathered tables
            TDT = {"f32": F32, "bf16": BF16, "f8": F8}[_TAB_NAME]
            n_tab = max(DBG_STEPS, 1)

            def lay_of(step):
                return LAYOUTS[STEP_LAYOUT[min(step, STEPS - 1)]]

            shard_bufs = [[DR.tile([lay_of(st)["nodes"][q] // ROW_N, ROW_W],
                                   TDT, name=f"m_shard{st}_{q}")
                           for q in range(lay_of(st)["n_ag"])]
                          for st in range(n_tab)]
            table_bufs = [[DR.tile([lay_of(st)["rows"][q] // ROW_N, ROW_W],
                                   TDT, addr_space="Shared",
                                   name=f"m_table{st}_{q}")
                           for q in range(lay_of(st)["n_ag"])]
                          for st in range(n_tab)]

            # ------- startup-critical loads first (xT feeds embedding) -----
            def load(dram_ap, shape, name, dt=BF16):
                tl = P.tile(shape, dt, name=name)
                nc.sync.dma_start(out=tl[:], in_=dram_ap)
                return tl

            xT_b = P.tile([FEAT, N_PAD], BF16, name="xT_b")
            nc.sync.dma_start(out=xT_b[:], in_=d_xT[:, :])
            wemb_b = load(d_wemb[:, :], [FEAT, H], "wemb")
            wmsg_b = [load(d_wmsg[s, :, :], [H, H], f"wmsg{s}")
                      for s in range(STEPS)]
            iota_f = P.tile([128, 128], F32, name="iota_f")
            nc.sync.dma_start(out=iota_f[:],
                              in_=d_iota.ap().to_broadcast([128, 128]))
            iota_b = P.tile([128, 128], BF16, name="iota_b")
            nc.vector.tensor_copy(iota_b[:], iota_f[:])
            ident_b = P.tile([128, 128], BF16, name="ident_b")
            nc.sync.dma_start(out=ident_b[:], in_=d_ident[:, :])

            # state
            h_t = P.tile([128, N_PAD], BF16, name="h_t")
            m_all = P.tile([128, N_PAD], TDT, name="m_all")
            aggT = P.tile([128, N_PAD], BF16, name="aggT")

            # warmup barrier: a tiny collective with no data deps issues at
            # kernel start, absorbing communicator bootstrap cost/skew before
            # the first real AllGather.
            d_warm_in = DR.tile([1, 2], F32, name="warm_in")
            d_warm_out = DR.tile([8, 2], F32, addr_space="Shared",
                                 name="warm_out")
            warm_t = P.tile([1, 2], F32, name="warm_t")
            nc.vector.memset(warm_t[:], 0.0)
            nc.sync.dma_start(out=d_warm_in[:, :], in_=warm_t[:])
            nc.gpsimd.collective_compute(
                "AllGather", mybir.AluOpType.bypass,
                ins=[d_warm_in.opt()], outs=[d_warm_out.opt()],
                replica_groups=[list(range(N_CORES))],
            )

            def msg_tile(t, step):
                pm = PS_M.tile([128, 128], F32, name="pm", tag="pm")
                nc.tensor.matmul(pm[:], lhsT=h_t[:, t * 128:(t + 1) * 128],
                                 rhs=wmsg_b[step % STEPS][:, :],
                                 start=True, stop=True)
                nc.scalar.activation(m_all[:, t * 128:(t + 1) * 128],
                                     pm[:], AF.Copy)

            def send_seg(step, q):
                """DMA m_all segment q to its shard and AllGather it.

                The shard write is split at 512-node chunk boundaries so each
                sub-DMA fires as soon as its chunk's messages are done - the
                collective then only waits on the last small piece."""
                lay = lay_of(step)
                shard = shard_bufs[step][q]
                n0, nn = lay["start"][q], lay["nodes"][q]
                cuts = [n0] + [b for b in range((n0 // 512 + 1) * 512,
                                                n0 + nn, 512)] + [n0 + nn]
                for lo, hi in zip(cuts[:-1], cuts[1:]):
                    src = m_all[:, lo:hi]
                    if ROW_N == 2:
                        # pair-row layout: row r = nodes (2r, 2r+1); node
                        # n=a*128+p lands at row a*64+p//2, offset (p%2)*128
                        out_ap = shard[(lo - n0) // 2:(hi - n0) // 2,
                                       :].rearrange(
                            "(a i) (e b) -> (i e) a b", i=64, e=2)
                    else:
                        out_ap = shard[lo - n0:hi - n0, :].rearrange(
                            "(a p) b -> p a b", p=128)
                    nc.sync.dma_start(
                        out=out_ap,
                        in_=src.rearrange("p (a b) -> p a b", b=128))
                if not DBG_NO_AG:
                    nc.gpsimd.collective_compute(
                        "AllGather", mybir.AluOpType.bypass,
                        ins=[shard.opt()],
                        outs=[table_bufs[step][q].opt()],
                        replica_groups=[list(range(N_CORES))],
                    )

            def ag_after_chunk_for(step):
                """chunk idx after which msg tiles for AG piece q are done"""
                lay = lay_of(step)
                m = {}
                for q in range(lay["n_ag"]):
                    last_tile = (lay["start"][q] + lay["nodes"][q]) // 128 - 1
                    m.setdefault(last_tile // 4, []).append(q)
                return m

            def gru_chunk(ch, step):
                off = ch * 512
                size = min(512, N_PAD - off)
                sl = slice(off, off + size)
                p_r = PS_GRU.tile([128, size], F32, name="p_r", tag="pgru")
                nc.tensor.matmul(p_r[:], lhsT=wih_b[:, 0:128],
                                 rhs=aggT[:, sl], start=True, stop=False)
                nc.tensor.matmul(p_r[:], lhsT=whh_b[:, 0:128],
                                 rhs=h_t[:, sl], start=False, stop=True)
                p_z = PS_GRU.tile([128, size], F32, name="p_z", tag="pgru")
                nc.tensor.matmul(p_z[:], lhsT=wih_b[:, 128:256],
                                 rhs=aggT[:, sl], start=True, stop=False)
                nc.tensor.matmul(p_z[:], lhsT=whh_b[:, 128:256],
                                 rhs=h_t[:, sl], start=False, stop=True)
                p_xn = PS_GRU.tile([128, size], F32, name="p_xn", tag="pgru")
                nc.tensor.matmul(p_xn[:], lhsT=wih_b[:, 256:384],
                                 rhs=aggT[:, sl], start=True, stop=True)
                p_hn = PS_GRU.tile([128, size], F32, name="p_hn", tag="pgru")
                nc.tensor.matmul(p_hn[:], lhsT=whh_b[:, 256:384],
                                 rhs=h_t[:, sl], start=True, stop=True)
                r_t = GP.tile([128, size], BF16, name="r_t", tag="gp1")
                nc.scalar.activation(r_t[:], p_r[:], AF.Sigmoid,
                                     bias=bsum[:, 0:1])
                z_t = GP.tile([128, size], BF16, name="z_t", tag="gp2")
                nc.scalar.activation(z_t[:], p_z[:], AF.Sigmoid,
                                     bias=bsum[:, 1:2])
                hn_t = GP.tile([128, size], BF16, name="hn_t", tag="gp3")
                nc.scalar.activation(hn_t[:], p_hn[:], AF.Identity,
                                     bias=bhh[:, 2:3])
                t1 = GP.tile([128, size], BF16, name="t1", tag="gp4")
                nc.vector.tensor_mul(t1[:], r_t[:], hn_t[:])
                u_t = GP.tile([128, size], F32, name="u_t", tag="gp5")
                nc.vector.tensor_add(u_t[:], t1[:], p_xn[:])
                n_t = GP.tile([128, size], F32, name="n_t", tag="gp6")
                nc.scalar.activation(n_t[:], u_t[:], AF.Tanh,
                                     bias=bih[:, 2:3])
                d_t = GP.tile([128, size], F32, name="d_t", tag="gp7")
                nc.vector.tensor_sub(d_t[:], h_t[:, sl], n_t[:])
                e_t = GP.tile([128, size], F32, name="e_t", tag="gp8")
                nc.vector.tensor_mul(e_t[:], z_t[:], d_t[:])
                nc.vector.tensor_add(h_t[:, sl], n_t[:], e_t[:])

            hr = P.tile([128, N_PAD], BF16, name="hr")
            pool_acc = P.tile([G_PADG, 128], F32, name="pool_acc")

            def readout_chunk(ch):
                off = ch * 512
                size = min(512, N_PAD - off)
                nc.scalar.activation(hr[:, off:off + size],
                                     h_t[:, off:off + size], AF.Relu)
                ts = list(range(ch * 4, min((ch + 1) * 4, NT)))
                pq = PS_M.tile([G_PADG, 128], F32, name="pq", tag="pq",
                               bufs=1)
                for i, t in enumerate(ts):
                    ptr2 = PS_M.tile([128, 128], BF16, name="ptr2", tag="pm")
                    nc.tensor.transpose(ptr2[:],
                                        hr[:, t * 128:(t + 1) * 128],
                                        ident_b[:])
                    hnm = GP.tile([128, 128], BF16, name="hnm", tag="gp1")
                    nc.scalar.activation(hnm[:], ptr2[:], AF.Copy)
                    nc.tensor.matmul(
                        pq[:], lhsT=gmat_b[:, t * G_PADG:(t + 1) * G_PADG],
                        rhs=hnm[:], start=(i == 0), stop=(i == len(ts) - 1))
                if ch == 0:
                    nc.scalar.activation(pool_acc[:], pq[:], AF.Copy)
                else:
                    nc.vector.tensor_add(pool_acc[:], pool_acc[:], pq[:])

            # ---------------- embedding ----------------
            for ch in range(N_CHUNKS):
                off = ch * 512
                size = min(512, N_PAD - off)
                pe = PS_GRU.tile([128, size], F32, name="pe_emb", tag="pgru")
                nc.tensor.matmul(pe[:], lhsT=wemb_b[:, :],
                                 rhs=xT_b[:, off:off + size],
                                 start=True, stop=True)
                nc.scalar.activation(h_t[:, off:off + size], pe[:], AF.Relu)

            # deferred loads: needed only after the first AllGather lands
            slot_tiles, ix_tiles = [], []
            for li, el in enumerate(els):
                st_t = P.tile([128, el["t_tot"]], SLOT_DT, name=f"slot_a{li}")
                nc.sync.dma_start(out=st_t[:], in_=d_slot[li][:, :])
                slot_tiles.append(st_t)
                ix_t = P.tile([128, el["t_tot"] * 8], I16, name=f"ix_a{li}")
                nc.sync.dma_start(out=ix_t[:], in_=d_idx[li][:, :])
                ix_tiles.append(ix_t)
            wih_b = load(d_wih[:, :], [H, 3 * H], "wih")
            whh_b = load(d_whh[:, :], [H, 3 * H], "whh")
            bih = load(d_bihT[:, :], [H, 3], "bih", F32)
            bhh = load(d_bhhT[:, :], [H, 3], "bhh", F32)
            bsum = P.tile([H, 3], F32, name="bsum")
            nc.vector.tensor_add(bsum[:], bih[:], bhh[:])
            w1_b = load(d_w1[:, :], [H, H], "w1")
            w2_b = load(d_w2[:, :], [H, 1], "w2")
            b1t = load(d_b1[:, :], [H, 1], "b1t", F32)
            b2t = load(d_b2[:, :], [1, 1], "b2t", F32)
            invc_t = load(d_invc[:, :], [G_PADG, 1], "invc_t", F32)
            gmat_b = load(d_gmat[:, :], [128, NT * G_PADG], "gmat")

            # ---------------- message-passing steps ----------------
            if DBG_STEPS > 0:
                lay0 = lay_of(0)
                # send each AllGather piece as soon as its tiles are done
                for q in range(lay0["n_ag"]):
                    t0q = lay0["start"][q] // 128
                    for t in range(t0q, t0q + lay0["tiles"][q]):
                        msg_tile(t, 0)
                    send_seg(0, q)

            for step in range(DBG_STEPS):
                lay = lay_of(step)
                el = els[STEP_LAYOUT[min(step, STEPS - 1)]]
                budget, tb = el["budget"], el["tb"]
                chunk_t0, chunk_nt = el["chunk_t0"], el["chunk_nt"]
                li = STEP_LAYOUT[min(step, STEPS - 1)]
                ix_all, slot_all = ix_tiles[li], slot_tiles[li]
                tab_half = [table_bufs[step][q][:, :]
                            for q in range(lay["n_ag"])]
                ag_after_chunk = (ag_after_chunk_for(step + 1)
                                  if step + 1 < DBG_STEPS else {})

                if DBG_NO_AGG:
                    nc.vector.memset(aggT[:], 0.0)
                for s in range(lay["n_ag"]):
                    last = s == lay["n_ag"] - 1
                    for ch in range(N_CHUNKS):
                        if not DBG_NO_AGG:
                            t0 = int(chunk_t0[ch, s])
                            n_ch = int(chunk_nt[ch, s])
                            if n_ch > 0:
                                E = EP.tile([128, n_ch, ROW_W], TDT,
                                            name="E", tag="E")
                                nc.gpsimd.dma_gather(
                                    E[:], tab_half[s],
                                    ix_all[:, t0 * 8:(t0 + n_ch) * 8],
                                    n_ch * 128, n_ch * 128, ROW_W,
                                    single_packet=bool(int(
                                        os.environ.get("K_SP", "0"))))
                            for g in _chunk_groups(ch):
                                tl = [(int(tb[g, s, p]) + kk, p)
                                      for p in range(ROW_N)
                                      for kk in range(int(budget[g, s, p]))]
                                if not tl:
                                    continue
                                pa = PS_AGG.tile([128, 128], F32, name="pa",
                                                 tag="pa")
                                # one is_equal builds the one-hot St for the
                                # whole cell (tiles are consecutive, so slot
                                # columns broadcast with stride-0 inner dim)
                                nb = len(tl)
                                tg0 = tl[0][0]
                                St_c = SP.tile([128, nb, 128], TDT,
                                               name="St", tag="St")
                                io = (iota_f if TAB_DT_F32 else iota_b)[:]
                                i_rep = bass.AP(
                                    io.tensor, io.offset,
                                    [tuple(io.ap[0]), (0, nb),
                                     tuple(io.ap[1])])
                                sl0 = slot_all[:, tg0:tg0 + 1]
                                s_rep = bass.AP(
                                    sl0.tensor, sl0.offset,
                                    [tuple(sl0.ap[0]), (1, nb), (0, 128)])
                                nc.vector.tensor_tensor(
                                    St_c[:], i_rep, s_rep,
                                    mybir.AluOpType.is_equal)
                                # DoubleRow fp8: two edge tiles per matmul
                                # (256-deep contraction) to halve PE SEQ work
                                if ROW_N == 2:
                                    pairs = [tl[j:j + 2]
                                             for j in range(0, len(tl), 2)]
                                else:
                                    pairs = [tl[j:j + 1]
                                             for j in range(len(tl))]
                                for j, pr in enumerate(pairs):
                                    st_fl = (j == 0, j == len(pairs) - 1)
                                    if len(pr) == 2:
                                        (ta, pa_), (tb_, pb_) = pr
                                        a0 = E[:, ta - t0,
                                               pa_ * 128:pa_ * 128 + 128]
                                        delta = ((tb_ - ta) * ROW_W
                                                 + (pb_ - pa_) * 128)
                                        lhsT2 = bass.AP(
                                            a0.tensor, a0.offset,
                                            [tuple(a0.ap[0]), (delta, 2),
                                             tuple(a0.ap[1])])
                                        nc.tensor.matmul(
                                            pa[:], lhsT=lhsT2,
                                            rhs=St_c[:, 2 * j:2 * j + 2, :],
                                            perf_mode=(mybir.MatmulPerfMode
                                                       .DoubleRow),
                                            start=st_fl[0], stop=st_fl[1])
                                    else:
                                        tg, p = pr[0]
                                        nc.tensor.matmul(
                                            pa[:],
                                            lhsT=E[:, tg - t0,
                                                   p * 128:(p + 1) * 128],
                                            rhs=St_c[:, tg - tg0, :],
                                            start=st_fl[0],
                                            stop=st_fl[1])
                                sl = slice(g * 128, (g + 1) * 128)
                                if s == 0:
                                    nc.scalar.activation(aggT[:, sl], pa[:],
                                                         AF.Copy)
                                else:
                                    nc.vector.tensor_add(aggT[:, sl],
                                                         aggT[:, sl], pa[:])
                        if last:
                            if not DBG_NO_GRU:
                                gru_chunk(ch, step)
                            if step + 1 < DBG_STEPS:
                                for t in range(ch * 4,
                                               min((ch + 1) * 4, NT)):
                                    msg_tile(t, step + 1)
                                for q in ag_after_chunk.get(ch, []):
                                    send_seg(step + 1, q)
                            else:
                                # final step: fold the readout (relu +
                                # transpose + pool matmul) into the chunk
                                # loop so the tail doesn't serialize
                                readout_chunk(ch)

            # ---------------- readout ----------------
            if DBG_STEPS == 0:
                for ch in range(N_CHUNKS):
                    readout_chunk(ch)
            # cross-core pool reduction: AllGather + local sum is cheaper
            # than AllReduce (no 1.875x collective penalty)
            d_pool_in = DR.tile([G_PADG, H], F32, name="pool_in")
            d_pool_out = DR.tile([N_CORES * G_PADG, H], F32,
                                 addr_space="Shared", name="pool_out")
            nc.sync.dma_start(out=d_pool_in[:, :], in_=pool_acc[:])
            nc.gpsimd.collective_compute(
                "AllGather", mybir.AluOpType.bypass,
                ins=[d_pool_in.opt()], outs=[d_pool_out.opt()],
                replica_groups=[list(range(N_CORES))],
            )
            pr8 = P.tile([G_PADG, N_CORES, 128], F32, name="pr8")
            nc.sync.dma_start(
                out=pr8[:],
                in_=d_pool_out.rearrange("(r g) f -> g r f", g=G_PADG))
            pool_r = P.tile([G_PADG, 128], F32, name="pool_r")
            nc.vector.tensor_add(pool_r[:], pr8[:, 0, :], pr8[:, 1, :])
            for r in range(2, N_CORES):
                nc.vector.tensor_add(pool_r[:], pool_r[:], pr8[:, r, :])
            pooled = P.tile([G_PADG, 128], BF16, name="pooled")
            nc.vector.tensor_scalar(pooled[:], pool_r[:], invc_t[:], None,
                                    mybir.AluOpType.mult)
            ppt = PS_M.tile([128, G_PADG], BF16, name="ppt", tag="pm")
            nc.tensor.transpose(ppt[:], pooled[:],
                                ident_b[0:G_PADG, 0:G_PADG])
            pooledT = P.tile([128, G_PADG], BF16, name="pooledT")
            nc.scalar.activation(pooledT[:], ppt[:], AF.Copy)
            pz1 = PS_M.tile([128, G_PADG], F32, name="pz1", tag="pm")
            nc.tensor.matmul(pz1[:], lhsT=w1_b[:, :], rhs=pooledT[:],
                             start=True, stop=True)
            z1 = P.tile([128, G_PADG], BF16, name="z1")
            nc.scalar.activation(z1[:], pz1[:], AF.Relu, bias=b1t[:, 0:1])
            po = PS_M.tile([1, G_PADG], F32, name="po", tag="pm")
            nc.tensor.matmul(po[:], lhsT=w2_b[:, :], rhs=z1[:],
                             start=True, stop=True)
            esb = P.tile([1, G_PADG], F32, name="esb")
            nc.scalar.activation(esb[:], po[:], AF.Exp, bias=b2t[:, 0:1])
            osb = P.tile([1, G_PADG], F32, name="osb")
            nc.scalar.activation(osb[:], esb[:], AF.Ln, bias=1.0)
            nc.sync.dma_start(out=d_out[:, :], in_=osb[:])

    nc.compile()
    return nc


# ----------------------------------------------------------------------------
# entry point
# ----------------------------------------------------------------------------

def make_in_maps(inputs, per_core, meta):
    return _make_in_maps(per_core, meta, **{
        k: inputs[k] for k in ("W_emb", "W_msg", "W_ih", "W_hh", "b_ih",
                               "b_hh", "W1", "b1", "W2", "b2")})


def _make_in_maps(per_core, meta, W_emb, W_msg, W_ih, W_hh, b_ih, b_hh,
                  W1, b1, W2, b2):
    bf = ml_dtypes.bfloat16
    shared = dict(
        iota=np.arange(128, dtype=np.float32).reshape(1, 128),
        ident=np.eye(128, dtype=np.float32).astype(bf),
        wemb=np.asarray(W_emb, np.float32).astype(bf),
        wmsg=np.asarray(W_msg, np.float32).astype(bf),
        wih=np.asarray(W_ih, np.float32).astype(bf),
        whh=np.asarray(W_hh, np.float32).astype(bf),
        bihT=np.ascontiguousarray(
            np.asarray(b_ih, np.float32).reshape(3, H).T),
        bhhT=np.ascontiguousarray(
            np.asarray(b_hh, np.float32).reshape(3, H).T),
        w1=np.asarray(W1, np.float32).astype(bf),
        b1=np.asarray(b1, np.float32).reshape(H, 1),
        w2=np.asarray(W2, np.float32).astype(bf),
        b2=np.asarray(b2, np.float32).reshape(1, 1),
        invc=meta["invc"],
    )
    in_maps = []
    for c in range(N_CORES):
        m = dict(shared)
        m["xT"] = per_core[c]["xT"]
        m["gmat"] = per_core[c]["gmat"]
        for li in range(len(LAYOUTS)):
            m[f"idx{li}"] = per_core[c][f"idx{li}"]
            m[f"slot{li}"] = per_core[c][f"slot{li}"]
        in_maps.append(m)
    return in_maps


def kernel(x, edge_index, batch, W_emb, W_msg, W_ih, W_hh, b_ih, b_hh,
           W1, b1, W2, b2):
    per_core, meta = _preprocess(x, edge_index, batch)
    nc = _build(meta)
    in_maps = _make_in_maps(per_core, meta, W_emb, W_msg, W_ih, W_hh,
                            b_ih, b_hh, W1, b1, W2, b2)

    trace = bool(int(os.environ.get("KERNEL_TRACE", "0")))
    res = run_bass_kernel_spmd(nc, in_maps, list(range(N_CORES)), trace=trace)
    LAST_RESULTS["exec_time_ns"] = res.exec_time_ns
    LAST_RESULTS["profile_json"] = res.profile_json
    LAST_RESULTS["nc"] = nc
    LAST_RESULTS["in_maps"] = in_maps

    return np.asarray(res.results[0]["out"][0, :N_GRAPHS], np.float32)



# revision 71
# speedup vs baseline: 1.0078x; 1.0078x over previous
"""Trainium2 Bass kernel for BondingGraphGNN (gnn_message_passing), v3.

Model (see reference):
  h = relu(x @ W_emb)
  4x: m = h @ W_msg[i]; agg = scatter_add(m[src] -> dst); h = GRU(agg, h)
  h = relu(h); pooled = segment_mean(h, batch); out = softplus(relu(pooled@W1+b1)@W2+b2)

Distribution: even node sharding (6250 nodes/core, padded). Per step each core
computes messages for its nodes, AllGathers the message table to DRAM, and
aggregates its incoming edges locally with a gather + one-hot-matmul
segment-sum, then runs the GRU.

v3 performance structure (~2.3x over v2 in the cost model):
- fp8e4 message table in pair-rows ([2 nodes, 256B] per row, parity-pure edge
  tiles pick their half via the lhsT offset): halves AllGather bytes; the
  scatter-sum averages ~16 messages so fp8 noise washes out (rel err ~1e-3).
- DoubleRow fp8 matmuls: two 128-edge tiles (256-deep contraction) per PE
  instruction via a custom strided lhsT AP - halves PE sequencer work, which
  otherwise bounds the aggregation.
- One DVE is_equal per (group, segment) cell builds the whole one-hot St
  stack (consecutive slot columns, stride-0 broadcast APs).
- Per-step AllGather in 3 pieces [16,17,17] tiles, consumed pass-by-pass
  (pass 0 initializes aggT, later passes accumulate). The small piece 0
  means the next step's first AllGather only waits on 4 GRU chunks, and the
  last pass's gathers hide under its flight - the collective engine streams
  nearly back-to-back.
- Step-invariant gather indices/slots preloaded once; startup reordered so
  the embedding's xT DMA precedes bulk prefetches; shard writes split at
  chunk boundaries so each fires as messages complete; readout folded into
  the last step's chunk loop; pool reduction via AllGather + local sum
  (cheaper than AllReduce); a tiny warmup AllGather absorbs communicator
  bootstrap before the first real collective.

Readout: per-core partial pooling + AllGather + local sum + tiny MLP
replicated on every core (host takes core 0).
"""

import os
import numpy as np

# the trimmed axon package in some containers lacks the NTFF profile hook
# module; stub it so run_bass_kernel_spmd(trace=True) degrades gracefully.
import sys as _sys, types as _types
try:
    import antenv.axon_hooks  # noqa: F401
except Exception:
    _m = _types.ModuleType("antenv.axon_hooks")
    _m.get_axon_ntff_profile_hook = lambda: None
    _sys.modules["antenv.axon_hooks"] = _m

import ml_dtypes
import concourse.bacc as bacc
import concourse.bass as bass
import concourse.mybir as mybir
import concourse.tile as tile
from concourse.bass_utils import run_bass_kernel_spmd

F32 = mybir.dt.float32
BF16 = mybir.dt.bfloat16
F8 = mybir.dt.float8e4
I16 = mybir.dt.int16
AF = mybir.ActivationFunctionType

N_NODES = 50000
N_EDGES = 800000
FEAT = 90
H = 128
STEPS = 4
N_GRAPHS = 100
N_CORES = 8

NC_NODES = N_NODES // N_CORES               # 6250 real nodes per core
N_PAD = 6400
NT = N_PAD // 128                           # dst groups per core (50)
N_CHUNKS = (NT + 3) // 4                    # 13 chunks of <=4 groups
G_PADG = 112                                # padded global graph count
PAD_SLOT = 255.0                            # sentinel slot -> all-zero S row

_TAB_NAME = os.environ.get("K_TAB", "f8")   # f32 | bf16 | f8
ROW_N = 2 if _TAB_NAME == "f8" else 1       # table nodes per row (fp8 pairs)
ROW_W = ROW_N * 128                         # table row width, elements


def _mk_layout(seg_tiles):
    """Per-step AllGather piece layout. Small first piece: the next step's
    first AllGather only waits on a few GRU chunks; small last piece: little
    post-collective aggregation work."""
    assert sum(seg_tiles) == NT
    nodes = [t * 128 for t in seg_tiles]
    start = [sum(nodes[:q]) for q in range(len(seg_tiles))]
    rows = [N_CORES * n for n in nodes]
    assert max(rows) // ROW_N <= 32768
    return dict(tiles=seg_tiles, nodes=nodes, start=start, rows=rows,
                n_ag=len(seg_tiles))


def _parse_segs(env, default):
    v = os.environ.get(env, "")
    return [int(x) for x in v.split(",")] if v else default


# per-step piece layouts (a distinct last-step layout is supported via
# K_SEGS_LAST but the uniform split benches best)
LAYOUTS = [_mk_layout(_parse_segs("K_SEGS", [16, 17, 17])),
           _mk_layout(_parse_segs("K_SEGS_LAST", [16, 17, 17]))]
STEP_LAYOUT = [0, 0, 0, 1]
if LAYOUTS[1]["tiles"] == LAYOUTS[0]["tiles"]:
    LAYOUTS = LAYOUTS[:1]
    STEP_LAYOUT = [0, 0, 0, 0]

TAB_DT_F32 = _TAB_NAME == "f32"
LAST_RESULTS = {}   # stash for test.py (exec time etc)


def _chunk_groups(ch):
    return range(ch * 4, min((ch + 1) * 4, NT))


# ----------------------------------------------------------------------------
# host-side layout
# ----------------------------------------------------------------------------

def _edge_layout(lay, d_core, grp, slot, s_core, s_local):
    """Tile/slot/idx tables for one AllGather piece layout."""
    n_seg = lay["n_ag"]
    bounds = np.asarray(lay["start"][1:] + [N_PAD], np.int64)
    seg = np.searchsorted(bounds, s_local, side="right")
    seg_nodes = np.asarray(lay["nodes"], np.int64)[seg]
    seg_start = np.asarray(lay["start"], np.int64)[seg]
    loc = s_core * seg_nodes + (s_local - seg_start)
    if ROW_N == 2:
        # fp8 pair-row table: row r of a piece holds sources (2r, 2r+1);
        # each tile is parity-pure so the matmul lhsT picks one half.
        par = loc % 2
        idxval = loc // 2
    else:
        par = np.zeros_like(loc)
        idxval = loc

    # per (core, grp, seg, par) counts -> uniform budgets
    cnt = np.zeros((N_CORES, NT, n_seg, ROW_N), np.int64)
    np.add.at(cnt, (d_core, grp, seg, par), 1)
    budget = np.ceil(cnt.max(axis=0) / 128).astype(np.int64)
    # every group needs >=1 pass-0 tile so the copy flush writes aggT
    need = budget[:, 0, :].sum(axis=1) == 0
    budget[need, 0, 0] = 1

    # tile order: seg-major, then chunk, then group, then parity
    tb = np.zeros((NT, n_seg, ROW_N), np.int64)
    chunk_t0 = np.zeros((N_CHUNKS, n_seg), np.int64)
    chunk_nt = np.zeros((N_CHUNKS, n_seg), np.int64)
    t = 0
    for s in range(n_seg):
        for ch in range(N_CHUNKS):
            chunk_t0[ch, s] = t
            for g in _chunk_groups(ch):
                for p in range(ROW_N):
                    tb[g, s, p] = t
                    t += int(budget[g, s, p])
            chunk_nt[ch, s] = t - chunk_t0[ch, s]
    t_tot = t

    # edge placement (vectorized)
    order = np.lexsort((par, grp, seg, d_core))
    sc = d_core[order]
    ss = seg[order]
    sg = grp[order]
    sp = par[order]
    sidx = idxval[order]
    sslot = slot[order]
    rid = ((sc * n_seg + ss) * NT + sg) * ROW_N + sp
    run_first = np.r_[0, np.flatnonzero(np.diff(rid)) + 1]
    run_len = np.diff(np.r_[run_first, len(rid)])
    k = np.arange(len(rid)) - np.repeat(run_first, run_len)
    tt = tb[sg, ss, sp] + k // 128
    pp = k % 128
    ct0 = chunk_t0[sg // 4, ss]
    pos = (tt - ct0) * 128 + pp

    idx_arr = np.zeros((N_CORES, 16, t_tot * 8), np.int16)
    slot_arr = np.full((N_CORES, 128, t_tot), PAD_SLOT, np.float32)
    idx_arr[sc, pos % 16, ct0 * 8 + pos // 16] = sidx.astype(np.int16)
    slot_arr[sc, pp, tt] = sslot
    return dict(budget=budget, tb=tb, chunk_t0=chunk_t0, chunk_nt=chunk_nt,
                t_tot=t_tot, idx_arr=idx_arr, slot_arr=slot_arr)


def _preprocess(x, edge_index, batch):
    batch = np.asarray(batch, np.int64)
    src = np.asarray(edge_index[0], np.int64)
    dst = np.asarray(edge_index[1], np.int64)
    frac = float(os.environ.get("K_EDGE_FRAC", "1"))
    if frac < 1.0:  # timing experiments only - wrong results
        n = int(len(src) * frac)
        src, dst = src[:n], dst[:n]

    d_core = dst // NC_NODES
    d_local = dst - d_core * NC_NODES
    grp = d_local // 128
    slot = (d_local % 128).astype(np.float32)
    s_core = src // NC_NODES
    s_local = src - s_core * NC_NODES

    els = [_edge_layout(lay, d_core, grp, slot, s_core, s_local)
           for lay in LAYOUTS]

    # per-core node features (transposed, padded, bf16) and graph one-hots
    counts = np.bincount(batch, minlength=N_GRAPHS).astype(np.float32)
    invc = np.zeros((G_PADG, 1), np.float32)
    invc[:N_GRAPHS, 0] = 1.0 / np.maximum(counts, 1.0)
    x = np.asarray(x, np.float32)
    slot_dt = np.float32 if TAB_DT_F32 else ml_dtypes.bfloat16
    per_core = []
    for c in range(N_CORES):
        n0 = c * NC_NODES
        xT = np.zeros((FEAT, N_PAD), np.float32)
        xT[:, :NC_NODES] = x[n0:n0 + NC_NODES].T
        gmat = np.zeros((128, NT * G_PADG), np.float32)
        l = np.arange(NC_NODES)
        gmat[l % 128, (l // 128) * G_PADG + batch[n0:n0 + NC_NODES]] = 1.0
        pc = dict(
            xT=xT.astype(ml_dtypes.bfloat16),
            gmat=gmat.astype(ml_dtypes.bfloat16),
        )
        for li, el in enumerate(els):
            pc[f"idx{li}"] = np.tile(el["idx_arr"][c], (8, 1))
            pc[f"slot{li}"] = el["slot_arr"][c].astype(slot_dt)
        per_core.append(pc)

    meta = dict(els=els, invc=invc)
    return per_core, meta


# ----------------------------------------------------------------------------
# device program
# ----------------------------------------------------------------------------

def _build(meta):
    DBG_STEPS = int(os.environ.get("K_STEPS", STEPS))
    DBG_NO_AG = bool(int(os.environ.get("K_NO_AG", "0")))
    DBG_NO_AGG = bool(int(os.environ.get("K_NO_AGG", "0")))
    DBG_NO_GRU = bool(int(os.environ.get("K_NO_GRU", "0")))
    els = meta["els"]

    nc = bacc.Bacc("TRN2", target_bir_lowering=False, debug=False,
                   num_devices=N_CORES)

    d_xT = nc.dram_tensor("xT", [FEAT, N_PAD], BF16, kind="ExternalInput")
    SLOT_DT = F32 if TAB_DT_F32 else BF16
    d_idx = [nc.dram_tensor(f"idx{li}", [128, el["t_tot"] * 8], I16,
                            kind="ExternalInput")
             for li, el in enumerate(els)]
    d_slot = [nc.dram_tensor(f"slot{li}", [128, el["t_tot"]], SLOT_DT,
                             kind="ExternalInput")
              for li, el in enumerate(els)]
    d_gmat = nc.dram_tensor("gmat", [128, NT * G_PADG], BF16,
                            kind="ExternalInput")
    d_invc = nc.dram_tensor("invc", [G_PADG, 1], F32, kind="ExternalInput")
    d_iota = nc.dram_tensor("iota", [1, 128], F32, kind="ExternalInput")
    d_ident = nc.dram_tensor("ident", [128, 128], BF16, kind="ExternalInput")
    d_wemb = nc.dram_tensor("wemb", [FEAT, H], BF16, kind="ExternalInput")
    d_wmsg = nc.dram_tensor("wmsg", [STEPS, H, H], BF16, kind="ExternalInput")
    d_wih = nc.dram_tensor("wih", [H, 3 * H], BF16, kind="ExternalInput")
    d_whh = nc.dram_tensor("whh", [H, 3 * H], BF16, kind="ExternalInput")
    d_bihT = nc.dram_tensor("bihT", [H, 3], F32, kind="ExternalInput")
    d_bhhT = nc.dram_tensor("bhhT", [H, 3], F32, kind="ExternalInput")
    d_w1 = nc.dram_tensor("w1", [H, H], BF16, kind="ExternalInput")
    d_b1 = nc.dram_tensor("b1", [H, 1], F32, kind="ExternalInput")
    d_w2 = nc.dram_tensor("w2", [H, 1], BF16, kind="ExternalInput")
    d_b2 = nc.dram_tensor("b2", [1, 1], F32, kind="ExternalInput")
    d_out = nc.dram_tensor("out", [1, G_PADG], F32, kind="ExternalOutput")

    with tile.TileContext(nc) as tc:
        with (
            tc.tile_pool(name="persist", bufs=1) as P,
            tc.tile_pool(name="dram", bufs=1, space="DRAM") as DR,
            tc.tile_pool(name="epool",
                         bufs=int(os.environ.get("K_EP", "4"))) as EP,
            tc.tile_pool(name="spool",
                         bufs=int(os.environ.get("K_SPOOL", "8"))) as SP,
            tc.tile_pool(name="gpool", bufs=2) as GP,
            tc.tile_pool(name="ps_agg", space="PSUM",
                         bufs=int(os.environ.get("K_PA", "2"))) as PS_AGG,
            tc.tile_pool(name="ps_m", bufs=1, space="PSUM") as PS_M,
            tc.tile_pool(name="ps_gru", space="PSUM",
                         bufs=int(os.environ.get("K_PG", "4"))) as PS_GRU,
        ):
            # DRAM temps: per-step message shards and gathered tables
            TDT = {"f32": F32, "bf16": BF16, "f8": F8}[_TAB_NAME]
            n_tab = max(DBG_STEPS, 1)

            def lay_of(step):
                return LAYOUTS[STEP_LAYOUT[min(step, STEPS - 1)]]

            shard_bufs = [[DR.tile([lay_of(st)["nodes"][q] // ROW_N, ROW_W],
                                   TDT, name=f"m_shard{st}_{q}")
                           for q in range(lay_of(st)["n_ag"])]
                          for st in range(n_tab)]
            table_bufs = [[DR.tile([lay_of(st)["rows"][q] // ROW_N, ROW_W],
                                   TDT, addr_space="Shared",
                                   name=f"m_table{st}_{q}")
                           for q in range(lay_of(st)["n_ag"])]
                          for st in range(n_tab)]

            # ------- startup-critical loads first (xT feeds embedding) -----
            def load(dram_ap, shape, name, dt=BF16):
                tl = P.tile(shape, dt, name=name)
                nc.sync.dma_start(out=tl[:], in_=dram_ap)
                return tl

            xT_b = P.tile([FEAT, N_PAD], BF16, name="xT_b")
            nc.sync.dma_start(out=xT_b[:], in_=d_xT[:, :])
            wemb_b = load(d_wemb[:, :], [FEAT, H], "wemb")
            wmsg_b = [load(d_wmsg[s, :, :], [H, H], f"wmsg{s}")
                      for s in range(STEPS)]
            iota_f = P.tile([128, 128], F32, name="iota_f")
            nc.sync.dma_start(out=iota_f[:],
                              in_=d_iota.ap().to_broadcast([128, 128]))
            iota_b = P.tile([128, 128], BF16, name="iota_b")
            nc.vector.tensor_copy(iota_b[:], iota_f[:])
            ident_b = P.tile([128, 128], BF16, name="ident_b")
            nc.sync.dma_start(out=ident_b[:], in_=d_ident[:, :])

            # state
            h_t = P.tile([128, N_PAD], BF16, name="h_t")
            m_all = P.tile([128, N_PAD], TDT, name="m_all")
            aggT = P.tile([128, N_PAD], BF16, name="aggT")

            # warmup barrier: a tiny collective with no data deps issues at
            # kernel start, absorbing communicator bootstrap cost/skew before
            # the first real AllGather.
            d_warm_in = DR.tile([1, 2], F32, name="warm_in")
            d_warm_out = DR.tile([8, 2], F32, addr_space="Shared",
                                 name="warm_out")
            warm_t = P.tile([1, 2], F32, name="warm_t")
            nc.vector.memset(warm_t[:], 0.0)
            nc.sync.dma_start(out=d_warm_in[:, :], in_=warm_t[:])
            nc.gpsimd.collective_compute(
                "AllGather", mybir.AluOpType.bypass,
                ins=[d_warm_in.opt()], outs=[d_warm_out.opt()],
                replica_groups=[list(range(N_CORES))],
            )

            def msg_tile(t, step):
                pm = PS_M.tile([128, 128], F32, name="pm", tag="pm")
                nc.tensor.matmul(pm[:], lhsT=h_t[:, t * 128:(t + 1) * 128],
                                 rhs=wmsg_b[step % STEPS][:, :],
                                 start=True, stop=True)
                nc.scalar.activation(m_all[:, t * 128:(t + 1) * 128],
                                     pm[:], AF.Copy)

            def send_seg(step, q):
                """DMA m_all segment q to its shard and AllGather it.

                The shard write is split at 512-node chunk boundaries so each
                sub-DMA fires as soon as its chunk's messages are done - the
                collective then only waits on the last small piece."""
                lay = lay_of(step)
                shard = shard_bufs[step][q]
                n0, nn = lay["start"][q], lay["nodes"][q]
                cuts = [n0] + [b for b in range((n0 // 512 + 1) * 512,
                                                n0 + nn, 512)] + [n0 + nn]
                for lo, hi in zip(cuts[:-1], cuts[1:]):
                    src = m_all[:, lo:hi]
                    if ROW_N == 2:
                        # pair-row layout: row r = nodes (2r, 2r+1); node
                        # n=a*128+p lands at row a*64+p//2, offset (p%2)*128
                        out_ap = shard[(lo - n0) // 2:(hi - n0) // 2,
                                       :].rearrange(
                            "(a i) (e b) -> (i e) a b", i=64, e=2)
                    else:
                        out_ap = shard[lo - n0:hi - n0, :].rearrange(
                            "(a p) b -> p a b", p=128)
                    nc.sync.dma_start(
                        out=out_ap,
                        in_=src.rearrange("p (a b) -> p a b", b=128))
                if not DBG_NO_AG:
                    nc.gpsimd.collective_compute(
                        "AllGather", mybir.AluOpType.bypass,
                        ins=[shard.opt()],
                        outs=[table_bufs[step][q].opt()],
                        replica_groups=[list(range(N_CORES))],
                    )

            def ag_after_chunk_for(step):
                """chunk idx after which msg tiles for AG piece q are done"""
                lay = lay_of(step)
                m = {}
                for q in range(lay["n_ag"]):
                    last_tile = (lay["start"][q] + lay["nodes"][q]) // 128 - 1
                    m.setdefault(last_tile // 4, []).append(q)
                return m

            def gru_chunk(ch, step):
                off = ch * 512
                size = min(512, N_PAD - off)
                sl = slice(off, off + size)
                p_r = PS_GRU.tile([128, size], F32, name="p_r", tag="pgru")
                nc.tensor.matmul(p_r[:], lhsT=wih_b[:, 0:128],
                                 rhs=aggT[:, sl], start=True, stop=False)
                nc.tensor.matmul(p_r[:], lhsT=whh_b[:, 0:128],
                                 rhs=h_t[:, sl], start=False, stop=True)
                p_z = PS_GRU.tile([128, size], F32, name="p_z", tag="pgru")
                nc.tensor.matmul(p_z[:], lhsT=wih_b[:, 128:256],
                                 rhs=aggT[:, sl], start=True, stop=False)
                nc.tensor.matmul(p_z[:], lhsT=whh_b[:, 128:256],
                                 rhs=h_t[:, sl], start=False, stop=True)
                p_xn = PS_GRU.tile([128, size], F32, name="p_xn", tag="pgru")
                nc.tensor.matmul(p_xn[:], lhsT=wih_b[:, 256:384],
                                 rhs=aggT[:, sl], start=True, stop=True)
                p_hn = PS_GRU.tile([128, size], F32, name="p_hn", tag="pgru")
                nc.tensor.matmul(p_hn[:], lhsT=whh_b[:, 256:384],
                                 rhs=h_t[:, sl], start=True, stop=True)
                r_t = GP.tile([128, size], BF16, name="r_t", tag="gp1")
                nc.scalar.activation(r_t[:], p_r[:], AF.Sigmoid,
                                     bias=bsum[:, 0:1])
                z_t = GP.tile([128, size], BF16, name="z_t", tag="gp2")
                nc.scalar.activation(z_t[:], p_z[:], AF.Sigmoid,
                                     bias=bsum[:, 1:2])
                hn_t = GP.tile([128, size], BF16, name="hn_t", tag="gp3")
                nc.scalar.activation(hn_t[:], p_hn[:], AF.Identity,
                                     bias=bhh[:, 2:3])
                t1 = GP.tile([128, size], BF16, name="t1", tag="gp4")
                nc.vector.tensor_mul(t1[:], r_t[:], hn_t[:])
                u_t = GP.tile([128, size], F32, name="u_t", tag="gp5")
                nc.vector.tensor_add(u_t[:], t1[:], p_xn[:])
                n_t = GP.tile([128, size], F32, name="n_t", tag="gp6")
                nc.scalar.activation(n_t[:], u_t[:], AF.Tanh,
                                     bias=bih[:, 2:3])
                d_t = GP.tile([128, size], F32, name="d_t", tag="gp7")
                nc.vector.tensor_sub(d_t[:], h_t[:, sl], n_t[:])
                e_t = GP.tile([128, size], F32, name="e_t", tag="gp8")
                nc.vector.tensor_mul(e_t[:], z_t[:], d_t[:])
                nc.vector.tensor_add(h_t[:, sl], n_t[:], e_t[:])

            hr = P.tile([128, N_PAD], BF16, name="hr")
            pool_acc = P.tile([G_PADG, 128], F32, name="pool_acc")

            def readout_chunk(ch):
                off = ch * 512
                size = min(512, N_PAD - off)
                nc.scalar.activation(hr[:, off:off + size],
                                     h_t[:, off:off + size], AF.Relu)
                ts = list(range(ch * 4, min((ch + 1) * 4, NT)))
                pq = PS_M.tile([G_PADG, 128], F32, name="pq", tag="pq",
                               bufs=1)
                for i, t in enumerate(ts):
                    ptr2 = PS_M.tile([128, 128], BF16, name="ptr2", tag="pm")
                    nc.tensor.transpose(ptr2[:],
                                        hr[:, t * 128:(t + 1) * 128],
                                        ident_b[:])
                    hnm = GP.tile([128, 128], BF16, name="hnm", tag="gp1")
                    nc.scalar.activation(hnm[:], ptr2[:], AF.Copy)
                    nc.tensor.matmul(
                        pq[:], lhsT=gmat_b[:, t * G_PADG:(t + 1) * G_PADG],
                        rhs=hnm[:], start=(i == 0), stop=(i == len(ts) - 1))
                if ch == 0:
                    nc.scalar.activation(pool_acc[:], pq[:], AF.Copy)
                else:
                    nc.vector.tensor_add(pool_acc[:], pool_acc[:], pq[:])

            def embed_chunk(ch):
                off = ch * 512
                size = min(512, N_PAD - off)
                pe = PS_GRU.tile([128, size], F32, name="pe_emb", tag="pgru")
                nc.tensor.matmul(pe[:], lhsT=wemb_b[:, :],
                                 rhs=xT_b[:, off:off + size],
                                 start=True, stop=True)
                nc.scalar.activation(h_t[:, off:off + size], pe[:], AF.Relu)

            # deferred loads: needed only after the first AllGather lands
            slot_tiles, ix_tiles = [], []
            for li, el in enumerate(els):
                st_t = P.tile([128, el["t_tot"]], SLOT_DT, name=f"slot_a{li}")
                nc.sync.dma_start(out=st_t[:], in_=d_slot[li][:, :])
                slot_tiles.append(st_t)
                ix_t = P.tile([128, el["t_tot"] * 8], I16, name=f"ix_a{li}")
                nc.sync.dma_start(out=ix_t[:], in_=d_idx[li][:, :])
                ix_tiles.append(ix_t)
            wih_b = load(d_wih[:, :], [H, 3 * H], "wih")
            whh_b = load(d_whh[:, :], [H, 3 * H], "whh")
            bih = load(d_bihT[:, :], [H, 3], "bih", F32)
            bhh = load(d_bhhT[:, :], [H, 3], "bhh", F32)
            bsum = P.tile([H, 3], F32, name="bsum")
            nc.vector.tensor_add(bsum[:], bih[:], bhh[:])
            w1_b = load(d_w1[:, :], [H, H], "w1")
            w2_b = load(d_w2[:, :], [H, 1], "w2")
            b1t = load(d_b1[:, :], [H, 1], "b1t", F32)
            b2t = load(d_b2[:, :], [1, 1], "b2t", F32)
            invc_t = load(d_invc[:, :], [G_PADG, 1], "invc_t", F32)
            gmat_b = load(d_gmat[:, :], [128, NT * G_PADG], "gmat")

            # ---------------- message-passing steps ----------------
            if DBG_STEPS > 0:
                lay0 = lay_of(0)
                # embed + message piece-by-piece so AllGather piece q is not
                # queued behind later chunks' embedding work
                emb_done = 0
                for q in range(lay0["n_ag"]):
                    t0q = lay0["start"][q] // 128
                    need_ch = (t0q + lay0["tiles"][q] + 3) // 4
                    while emb_done < min(need_ch, N_CHUNKS):
                        embed_chunk(emb_done)
                        emb_done += 1
                    for t in range(t0q, t0q + lay0["tiles"][q]):
                        msg_tile(t, 0)
                    send_seg(0, q)
                while emb_done < N_CHUNKS:
                    embed_chunk(emb_done)
                    emb_done += 1
            else:
                for ch in range(N_CHUNKS):
                    embed_chunk(ch)

            for step in range(DBG_STEPS):
                lay = lay_of(step)
                el = els[STEP_LAYOUT[min(step, STEPS - 1)]]
                budget, tb = el["budget"], el["tb"]
                chunk_t0, chunk_nt = el["chunk_t0"], el["chunk_nt"]
                li = STEP_LAYOUT[min(step, STEPS - 1)]
                ix_all, slot_all = ix_tiles[li], slot_tiles[li]
                tab_half = [table_bufs[step][q][:, :]
                            for q in range(lay["n_ag"])]
                ag_after_chunk = (ag_after_chunk_for(step + 1)
                                  if step + 1 < DBG_STEPS else {})

                if DBG_NO_AGG:
                    nc.vector.memset(aggT[:], 0.0)
                for s in range(lay["n_ag"]):
                    last = s == lay["n_ag"] - 1
                    for ch in range(N_CHUNKS):
                        if not DBG_NO_AGG:
                            t0 = int(chunk_t0[ch, s])
                            n_ch = int(chunk_nt[ch, s])
                            if n_ch > 0:
                                E = EP.tile([128, n_ch, ROW_W], TDT,
                                            name="E", tag="E")
                                nc.gpsimd.dma_gather(
                                    E[:], tab_half[s],
                                    ix_all[:, t0 * 8:(t0 + n_ch) * 8],
                                    n_ch * 128, n_ch * 128, ROW_W,
                                    single_packet=bool(int(
                                        os.environ.get("K_SP", "0"))))
                            for g in _chunk_groups(ch):
                                tl = [(int(tb[g, s, p]) + kk, p)
                                      for p in range(ROW_N)
                                      for kk in range(int(budget[g, s, p]))]
                                if not tl:
                                    continue
                                pa = PS_AGG.tile([128, 128], F32, name="pa",
                                                 tag="pa")
                                # one is_equal builds the one-hot St for the
                                # whole cell (tiles are consecutive, so slot
                                # columns broadcast with stride-0 inner dim)
                                nb = len(tl)
                                tg0 = tl[0][0]
                                St_c = SP.tile([128, nb, 128], TDT,
                                               name="St", tag="St")
                                io = (iota_f if TAB_DT_F32 else iota_b)[:]
                                i_rep = bass.AP(
                                    io.tensor, io.offset,
                                    [tuple(io.ap[0]), (0, nb),
                                     tuple(io.ap[1])])
                                sl0 = slot_all[:, tg0:tg0 + 1]
                                s_rep = bass.AP(
                                    sl0.tensor, sl0.offset,
                                    [tuple(sl0.ap[0]), (1, nb), (0, 128)])
                                nc.vector.tensor_tensor(
                                    St_c[:], i_rep, s_rep,
                                    mybir.AluOpType.is_equal)
                                # DoubleRow fp8: two edge tiles per matmul
                                # (256-deep contraction) to halve PE SEQ work
                                if ROW_N == 2:
                                    pairs = [tl[j:j + 2]
                                             for j in range(0, len(tl), 2)]
                                else:
                                    pairs = [tl[j:j + 1]
                                             for j in range(len(tl))]
                                for j, pr in enumerate(pairs):
                                    st_fl = (j == 0, j == len(pairs) - 1)
                                    if len(pr) == 2:
                                        (ta, pa_), (tb_, pb_) = pr
                                        a0 = E[:, ta - t0,
                                               pa_ * 128:pa_ * 128 + 128]
                                        delta = ((tb_ - ta) * ROW_W
                                                 + (pb_ - pa_) * 128)
                                        lhsT2 = bass.AP(
                                            a0.tensor, a0.offset,
                                            [tuple(a0.ap[0]), (delta, 2),
                                             tuple(a0.ap[1])])
                                        nc.tensor.matmul(
                                            pa[:], lhsT=lhsT2,
                                            rhs=St_c[:, 2 * j:2 * j + 2, :],
                                            perf_mode=(mybir.MatmulPerfMode
                                                       .DoubleRow),
                                            start=st_fl[0], stop=st_fl[1])
                                    else:
                                        tg, p = pr[0]
                                        nc.tensor.matmul(
                                            pa[:],
                                            lhsT=E[:, tg - t0,
                                                   p * 128:(p + 1) * 128],
                                            rhs=St_c[:, tg - tg0, :],
                                            start=st_fl[0],
                                            stop=st_fl[1])
                                sl = slice(g * 128, (g + 1) * 128)
                                if s == 0:
                                    nc.scalar.activation(aggT[:, sl], pa[:],
                                                         AF.Copy)
                                else:
                                    nc.vector.tensor_add(aggT[:, sl],
                                                         aggT[:, sl], pa[:])
                        if last:
                            if not DBG_NO_GRU:
                                gru_chunk(ch, step)
                            if step + 1 < DBG_STEPS:
                                for t in range(ch * 4,
                                               min((ch + 1) * 4, NT)):
                                    msg_tile(t, step + 1)
                                for q in ag_after_chunk.get(ch, []):
                                    send_seg(step + 1, q)
                            else:
                                # final step: fold the readout (relu +
                                # transpose + pool matmul) into the chunk
                                # loop so the tail doesn't serialize
                                readout_chunk(ch)

            # ---------------- readout ----------------
            if DBG_STEPS == 0:
                for ch in range(N_CHUNKS):
                    readout_chunk(ch)
            # cross-core pool reduction: AllGather + local sum is cheaper
            # than AllReduce (no 1.875x collective penalty); bf16 partials
            # halve the collective bytes (pooled means tolerate the rounding)
            pool_bf = P.tile([G_PADG, 128], BF16, name="pool_bf")
            nc.vector.tensor_copy(pool_bf[:], pool_acc[:])
            d_pool_in = DR.tile([G_PADG, H], BF16, name="pool_in")
            d_pool_out = DR.tile([N_CORES * G_PADG, H], BF16,
                                 addr_space="Shared", name="pool_out")
            nc.sync.dma_start(out=d_pool_in[:, :], in_=pool_bf[:])
            nc.gpsimd.collective_compute(
                "AllGather", mybir.AluOpType.bypass,
                ins=[d_pool_in.opt()], outs=[d_pool_out.opt()],
                replica_groups=[list(range(N_CORES))],
            )
            pr8 = P.tile([G_PADG, N_CORES, 128], BF16, name="pr8")
            nc.sync.dma_start(
                out=pr8[:],
                in_=d_pool_out.rearrange("(r g) f -> g r f", g=G_PADG))
            pool_r = P.tile([G_PADG, 128], F32, name="pool_r")
            nc.vector.tensor_add(pool_r[:], pr8[:, 0, :], pr8[:, 1, :])
            for r in range(2, N_CORES):
                nc.vector.tensor_add(pool_r[:], pool_r[:], pr8[:, r, :])
            pooled = P.tile([G_PADG, 128], BF16, name="pooled")
            nc.vector.tensor_scalar(pooled[:], pool_r[:], invc_t[:], None,
                                    mybir.AluOpType.mult)
            ppt = PS_M.tile([128, G_PADG], BF16, name="ppt", tag="pm")
            nc.tensor.transpose(ppt[:], pooled[:],
                                ident_b[0:G_PADG, 0:G_PADG])
            pooledT = P.tile([128, G_PADG], BF16, name="pooledT")
            nc.scalar.activation(pooledT[:], ppt[:], AF.Copy)
            pz1 = PS_M.tile([128, G_PADG], F32, name="pz1", tag="pm")
            nc.tensor.matmul(pz1[:], lhsT=w1_b[:, :], rhs=pooledT[:],
                             start=True, stop=True)
            z1 = P.tile([128, G_PADG], BF16, name="z1")
            nc.scalar.activation(z1[:], pz1[:], AF.Relu, bias=b1t[:, 0:1])
            po = PS_M.tile([1, G_PADG], F32, name="po", tag="pm")
            nc.tensor.matmul(po[:], lhsT=w2_b[:, :], rhs=z1[:],
                             start=True, stop=True)
            esb = P.tile([1, G_PADG], F32, name="esb")
            nc.scalar.activation(esb[:], po[:], AF.Exp, bias=b2t[:, 0:1])
            osb = P.tile([1, G_PADG], F32, name="osb")
            nc.scalar.activation(osb[:], esb[:], AF.Ln, bias=1.0)
            nc.sync.dma_start(out=d_out[:, :], in_=osb[:])

    nc.compile()
    return nc


# ----------------------------------------------------------------------------
# entry point
# ----------------------------------------------------------------------------

def make_in_maps(inputs, per_core, meta):
    return _make_in_maps(per_core, meta, **{
        k: inputs[k] for k in ("W_emb", "W_msg", "W_ih", "W_hh", "b_ih",
                               "b_hh", "W1", "b1", "W2", "b2")})


def _make_in_maps(per_core, meta, W_emb, W_msg, W_ih, W_hh, b_ih, b_hh,
                  W1, b1, W2, b2):
    bf = ml_dtypes.bfloat16
    shared = dict(
        iota=np.arange(128, dtype=np.float32).reshape(1, 128),
        ident=np.eye(128, dtype=np.float32).astype(bf),
        wemb=np.asarray(W_emb, np.float32).astype(bf),
        wmsg=np.asarray(W_msg, np.float32).astype(bf),
        wih=np.asarray(W_ih, np.float32).astype(bf),
        whh=np.asarray(W_hh, np.float32).astype(bf),
        bihT=np.ascontiguousarray(
            np.asarray(b_ih, np.float32).reshape(3, H).T),
        bhhT=np.ascontiguousarray(
            np.asarray(b_hh, np.float32).reshape(3, H).T),
        w1=np.asarray(W1, np.float32).astype(bf),
        b1=np.asarray(b1, np.float32).reshape(H, 1),
        w2=np.asarray(W2, np.float32).astype(bf),
        b2=np.asarray(b2, np.float32).reshape(1, 1),
        invc=meta["invc"],
    )
    in_maps = []
    for c in range(N_CORES):
        m = dict(shared)
        m["xT"] = per_core[c]["xT"]
        m["gmat"] = per_core[c]["gmat"]
        for li in range(len(LAYOUTS)):
            m[f"idx{li}"] = per_core[c][f"idx{li}"]
            m[f"slot{li}"] = per_core[c][f"slot{li}"]
        in_maps.append(m)
    return in_maps


def kernel(x, edge_index, batch, W_emb, W_msg, W_ih, W_hh, b_ih, b_hh,
           W1, b1, W2, b2):
    per_core, meta = _preprocess(x, edge_index, batch)
    nc = _build(meta)
    in_maps = _make_in_maps(per_core, meta, W_emb, W_msg, W_ih, W_hh,
                            b_ih, b_hh, W1, b1, W2, b2)

    trace = bool(int(os.environ.get("KERNEL_TRACE", "0")))
    res = run_bass_kernel_spmd(nc, in_maps, list(range(N_CORES)), trace=trace)
    LAST_RESULTS["exec_time_ns"] = res.exec_time_ns
    LAST_RESULTS["profile_json"] = res.profile_json
    LAST_RESULTS["nc"] = nc
    LAST_RESULTS["in_maps"] = in_maps

    return np.asarray(res.results[0]["out"][0, :N_GRAPHS], np.float32)



# revision 72
# speedup vs baseline: 1.0177x; 1.0098x over previous
"""Trainium2 Bass kernel for BondingGraphGNN (gnn_message_passing), v3.

Model (see reference):
  h = relu(x @ W_emb)
  4x: m = h @ W_msg[i]; agg = scatter_add(m[src] -> dst); h = GRU(agg, h)
  h = relu(h); pooled = segment_mean(h, batch); out = softplus(relu(pooled@W1+b1)@W2+b2)

Distribution: even node sharding (6250 nodes/core, padded). Per step each core
computes messages for its nodes, AllGathers the message table to DRAM, and
aggregates its incoming edges locally with a gather + one-hot-matmul
segment-sum, then runs the GRU.

v3 performance structure (~2.3x over v2 in the cost model):
- fp8e4 message table in pair-rows ([2 nodes, 256B] per row, parity-pure edge
  tiles pick their half via the lhsT offset): halves AllGather bytes; the
  scatter-sum averages ~16 messages so fp8 noise washes out (rel err ~1e-3).
- DoubleRow fp8 matmuls: two 128-edge tiles (256-deep contraction) per PE
  instruction via a custom strided lhsT AP - halves PE sequencer work, which
  otherwise bounds the aggregation.
- One DVE is_equal per (group, segment) cell builds the whole one-hot St
  stack (consecutive slot columns, stride-0 broadcast APs).
- Per-step AllGather in 3 pieces [16,17,17] tiles, consumed pass-by-pass
  (pass 0 initializes aggT, later passes accumulate). The small piece 0
  means the next step's first AllGather only waits on 4 GRU chunks, and the
  last pass's gathers hide under its flight - the collective engine streams
  nearly back-to-back.
- Step-invariant gather indices/slots preloaded once; startup reordered so
  the embedding's xT DMA precedes bulk prefetches; shard writes split at
  chunk boundaries so each fires as messages complete; readout folded into
  the last step's chunk loop; pool reduction via AllGather + local sum
  (cheaper than AllReduce); a tiny warmup AllGather absorbs communicator
  bootstrap before the first real collective.

Readout: per-core partial pooling + AllGather + local sum + tiny MLP
replicated on every core (host takes core 0).
"""

import os
import numpy as np

# the trimmed axon package in some containers lacks the NTFF profile hook
# module; stub it so run_bass_kernel_spmd(trace=True) degrades gracefully.
import sys as _sys, types as _types
try:
    import antenv.axon_hooks  # noqa: F401
except Exception:
    _m = _types.ModuleType("antenv.axon_hooks")
    _m.get_axon_ntff_profile_hook = lambda: None
    _sys.modules["antenv.axon_hooks"] = _m

import ml_dtypes
import concourse.bacc as bacc
import concourse.bass as bass
import concourse.mybir as mybir
import concourse.tile as tile
from concourse.bass_utils import run_bass_kernel_spmd

F32 = mybir.dt.float32
BF16 = mybir.dt.bfloat16
F8 = mybir.dt.float8e4
I16 = mybir.dt.int16
AF = mybir.ActivationFunctionType

N_NODES = 50000
N_EDGES = 800000
FEAT = 90
H = 128
STEPS = 4
N_GRAPHS = 100
N_CORES = 8

NC_NODES = N_NODES // N_CORES               # 6250 real nodes per core
N_PAD = 6400
NT = N_PAD // 128                           # dst groups per core (50)
N_CHUNKS = (NT + 3) // 4                    # 13 chunks of <=4 groups
G_PADG = 112                                # padded global graph count
PAD_SLOT = 255.0                            # sentinel slot -> all-zero S row

_TAB_NAME = os.environ.get("K_TAB", "f8")   # f32 | bf16 | f8
ROW_N = 2 if _TAB_NAME == "f8" else 1       # table nodes per row (fp8 pairs)
ROW_W = ROW_N * 128                         # table row width, elements


def _mk_layout(seg_tiles):
    """Per-step AllGather piece layout. Small first piece: the next step's
    first AllGather only waits on a few GRU chunks; small last piece: little
    post-collective aggregation work."""
    assert sum(seg_tiles) == NT
    nodes = [t * 128 for t in seg_tiles]
    start = [sum(nodes[:q]) for q in range(len(seg_tiles))]
    rows = [N_CORES * n for n in nodes]
    assert max(rows) // ROW_N <= 32768
    return dict(tiles=seg_tiles, nodes=nodes, start=start, rows=rows,
                n_ag=len(seg_tiles))


def _parse_segs(env, default):
    v = os.environ.get(env, "")
    return [int(x) for x in v.split(",")] if v else default


# per-step piece layouts (a distinct last-step layout is supported via
# K_SEGS_LAST but the uniform split benches best)
LAYOUTS = [_mk_layout(_parse_segs("K_SEGS", [16, 17, 17])),
           _mk_layout(_parse_segs("K_SEGS_LAST", [16, 17, 17]))]
STEP_LAYOUT = [0, 0, 0, 1]
if LAYOUTS[1]["tiles"] == LAYOUTS[0]["tiles"]:
    LAYOUTS = LAYOUTS[:1]
    STEP_LAYOUT = [0, 0, 0, 0]

TAB_DT_F32 = _TAB_NAME == "f32"
LAST_RESULTS = {}   # stash for test.py (exec time etc)


def _chunk_groups(ch):
    return range(ch * 4, min((ch + 1) * 4, NT))


# ----------------------------------------------------------------------------
# host-side layout
# ----------------------------------------------------------------------------

def _edge_layout(lay, d_core, grp, slot, s_core, s_local):
    """Tile/slot/idx tables for one AllGather piece layout."""
    n_seg = lay["n_ag"]
    bounds = np.asarray(lay["start"][1:] + [N_PAD], np.int64)
    seg = np.searchsorted(bounds, s_local, side="right")
    seg_nodes = np.asarray(lay["nodes"], np.int64)[seg]
    seg_start = np.asarray(lay["start"], np.int64)[seg]
    loc = s_core * seg_nodes + (s_local - seg_start)
    if ROW_N == 2:
        # fp8 pair-row table: row r of a piece holds sources (2r, 2r+1);
        # each tile is parity-pure so the matmul lhsT picks one half.
        par = loc % 2
        idxval = loc // 2
    else:
        par = np.zeros_like(loc)
        idxval = loc

    # per (core, grp, seg, par) counts -> uniform budgets
    cnt = np.zeros((N_CORES, NT, n_seg, ROW_N), np.int64)
    np.add.at(cnt, (d_core, grp, seg, par), 1)
    budget = np.ceil(cnt.max(axis=0) / 128).astype(np.int64)
    # every group needs >=1 pass-0 tile so the copy flush writes aggT
    need = budget[:, 0, :].sum(axis=1) == 0
    budget[need, 0, 0] = 1

    # tile order: seg-major, then chunk, then group, then parity
    tb = np.zeros((NT, n_seg, ROW_N), np.int64)
    chunk_t0 = np.zeros((N_CHUNKS, n_seg), np.int64)
    chunk_nt = np.zeros((N_CHUNKS, n_seg), np.int64)
    t = 0
    for s in range(n_seg):
        for ch in range(N_CHUNKS):
            chunk_t0[ch, s] = t
            for g in _chunk_groups(ch):
                for p in range(ROW_N):
                    tb[g, s, p] = t
                    t += int(budget[g, s, p])
            chunk_nt[ch, s] = t - chunk_t0[ch, s]
    t_tot = t

    # edge placement (vectorized)
    order = np.lexsort((par, grp, seg, d_core))
    sc = d_core[order]
    ss = seg[order]
    sg = grp[order]
    sp = par[order]
    sidx = idxval[order]
    sslot = slot[order]
    rid = ((sc * n_seg + ss) * NT + sg) * ROW_N + sp
    run_first = np.r_[0, np.flatnonzero(np.diff(rid)) + 1]
    run_len = np.diff(np.r_[run_first, len(rid)])
    k = np.arange(len(rid)) - np.repeat(run_first, run_len)
    tt = tb[sg, ss, sp] + k // 128
    pp = k % 128
    ct0 = chunk_t0[sg // 4, ss]
    pos = (tt - ct0) * 128 + pp

    idx_arr = np.zeros((N_CORES, 16, t_tot * 8), np.int16)
    slot_arr = np.full((N_CORES, 128, t_tot), PAD_SLOT, np.float32)
    idx_arr[sc, pos % 16, ct0 * 8 + pos // 16] = sidx.astype(np.int16)
    slot_arr[sc, pp, tt] = sslot
    return dict(budget=budget, tb=tb, chunk_t0=chunk_t0, chunk_nt=chunk_nt,
                t_tot=t_tot, idx_arr=idx_arr, slot_arr=slot_arr)


def _preprocess(x, edge_index, batch):
    batch = np.asarray(batch, np.int64)
    src = np.asarray(edge_index[0], np.int64)
    dst = np.asarray(edge_index[1], np.int64)
    frac = float(os.environ.get("K_EDGE_FRAC", "1"))
    if frac < 1.0:  # timing experiments only - wrong results
        n = int(len(src) * frac)
        src, dst = src[:n], dst[:n]

    d_core = dst // NC_NODES
    d_local = dst - d_core * NC_NODES
    grp = d_local // 128
    slot = (d_local % 128).astype(np.float32)
    s_core = src // NC_NODES
    s_local = src - s_core * NC_NODES

    els = [_edge_layout(lay, d_core, grp, slot, s_core, s_local)
           for lay in LAYOUTS]

    # per-core node features (transposed, padded, bf16) and graph one-hots
    counts = np.bincount(batch, minlength=N_GRAPHS).astype(np.float32)
    invc = np.zeros((G_PADG, 1), np.float32)
    invc[:N_GRAPHS, 0] = 1.0 / np.maximum(counts, 1.0)
    x = np.asarray(x, np.float32)
    slot_dt = np.float32 if TAB_DT_F32 else ml_dtypes.bfloat16
    per_core = []
    for c in range(N_CORES):
        n0 = c * NC_NODES
        xT = np.zeros((FEAT, N_PAD), np.float32)
        xT[:, :NC_NODES] = x[n0:n0 + NC_NODES].T
        gmat = np.zeros((128, NT * G_PADG), np.float32)
        l = np.arange(NC_NODES)
        gmat[l % 128, (l // 128) * G_PADG + batch[n0:n0 + NC_NODES]] = 1.0
        pc = dict(
            xT=xT.astype(ml_dtypes.bfloat16),
            gmat=gmat.astype(ml_dtypes.bfloat16),
        )
        for li, el in enumerate(els):
            pc[f"idx{li}"] = np.tile(el["idx_arr"][c], (8, 1))
            pc[f"slot{li}"] = el["slot_arr"][c].astype(slot_dt)
        per_core.append(pc)

    meta = dict(els=els, invc=invc)
    return per_core, meta


# ----------------------------------------------------------------------------
# device program
# ----------------------------------------------------------------------------

def _build(meta):
    DBG_STEPS = int(os.environ.get("K_STEPS", STEPS))
    DBG_NO_AG = bool(int(os.environ.get("K_NO_AG", "0")))
    DBG_NO_AGG = bool(int(os.environ.get("K_NO_AGG", "0")))
    DBG_NO_GRU = bool(int(os.environ.get("K_NO_GRU", "0")))
    els = meta["els"]

    nc = bacc.Bacc("TRN2", target_bir_lowering=False, debug=False,
                   num_devices=N_CORES)

    d_xT = nc.dram_tensor("xT", [FEAT, N_PAD], BF16, kind="ExternalInput")
    SLOT_DT = F32 if TAB_DT_F32 else BF16
    d_idx = [nc.dram_tensor(f"idx{li}", [128, el["t_tot"] * 8], I16,
                            kind="ExternalInput")
             for li, el in enumerate(els)]
    d_slot = [nc.dram_tensor(f"slot{li}", [128, el["t_tot"]], SLOT_DT,
                             kind="ExternalInput")
              for li, el in enumerate(els)]
    d_gmat = nc.dram_tensor("gmat", [128, NT * G_PADG], BF16,
                            kind="ExternalInput")
    d_invc = nc.dram_tensor("invc", [G_PADG, 1], F32, kind="ExternalInput")
    d_iota = nc.dram_tensor("iota", [1, 128], F32, kind="ExternalInput")
    d_ident = nc.dram_tensor("ident", [128, 128], BF16, kind="ExternalInput")
    d_wemb = nc.dram_tensor("wemb", [FEAT, H], BF16, kind="ExternalInput")
    d_wmsg = nc.dram_tensor("wmsg", [STEPS, H, H], BF16, kind="ExternalInput")
    d_wih = nc.dram_tensor("wih", [H, 3 * H], BF16, kind="ExternalInput")
    d_whh = nc.dram_tensor("whh", [H, 3 * H], BF16, kind="ExternalInput")
    d_bihT = nc.dram_tensor("bihT", [H, 3], F32, kind="ExternalInput")
    d_bhhT = nc.dram_tensor("bhhT", [H, 3], F32, kind="ExternalInput")
    d_w1 = nc.dram_tensor("w1", [H, H], BF16, kind="ExternalInput")
    d_b1 = nc.dram_tensor("b1", [H, 1], F32, kind="ExternalInput")
    d_w2 = nc.dram_tensor("w2", [H, 1], BF16, kind="ExternalInput")
    d_b2 = nc.dram_tensor("b2", [1, 1], F32, kind="ExternalInput")
    d_out = nc.dram_tensor("out", [1, G_PADG], F32, kind="ExternalOutput")

    with tile.TileContext(nc) as tc:
        with (
            tc.tile_pool(name="persist", bufs=1) as P,
            tc.tile_pool(name="dram", bufs=1, space="DRAM") as DR,
            tc.tile_pool(name="epool",
                         bufs=int(os.environ.get("K_EP", "4"))) as EP,
            tc.tile_pool(name="spool",
                         bufs=int(os.environ.get("K_SPOOL", "8"))) as SP,
            tc.tile_pool(name="gpool", bufs=2) as GP,
            tc.tile_pool(name="ps_agg", space="PSUM",
                         bufs=int(os.environ.get("K_PA", "2"))) as PS_AGG,
            tc.tile_pool(name="ps_m", bufs=1, space="PSUM") as PS_M,
            tc.tile_pool(name="ps_gru", space="PSUM",
                         bufs=int(os.environ.get("K_PG", "4"))) as PS_GRU,
        ):
            # DRAM temps: per-step message shards and gathered tables
            TDT = {"f32": F32, "bf16": BF16, "f8": F8}[_TAB_NAME]
            n_tab = max(DBG_STEPS, 1)

            def lay_of(step):
                return LAYOUTS[STEP_LAYOUT[min(step, STEPS - 1)]]

            shard_bufs = [[DR.tile([lay_of(st)["nodes"][q] // ROW_N, ROW_W],
                                   TDT, name=f"m_shard{st}_{q}")
                           for q in range(lay_of(st)["n_ag"])]
                          for st in range(n_tab)]
            table_bufs = [[DR.tile([lay_of(st)["rows"][q] // ROW_N, ROW_W],
                                   TDT, addr_space="Shared",
                                   name=f"m_table{st}_{q}")
                           for q in range(lay_of(st)["n_ag"])]
                          for st in range(n_tab)]

            # ------- startup-critical loads first (xT feeds embedding) -----
            def load(dram_ap, shape, name, dt=BF16):
                tl = P.tile(shape, dt, name=name)
                nc.sync.dma_start(out=tl[:], in_=dram_ap)
                return tl

            xT_b = P.tile([FEAT, N_PAD], BF16, name="xT_b")
            nc.sync.dma_start(out=xT_b[:], in_=d_xT[:, :])
            wemb_b = load(d_wemb[:, :], [FEAT, H], "wemb")
            wmsg_b = [load(d_wmsg[s, :, :], [H, H], f"wmsg{s}")
                      for s in range(STEPS)]
            iota_f = P.tile([128, 128], F32, name="iota_f")
            nc.sync.dma_start(out=iota_f[:],
                              in_=d_iota.ap().to_broadcast([128, 128]))
            iota_b = P.tile([128, 128], BF16, name="iota_b")
            nc.vector.tensor_copy(iota_b[:], iota_f[:])
            ident_b = P.tile([128, 128], BF16, name="ident_b")
            nc.sync.dma_start(out=ident_b[:], in_=d_ident[:, :])

            # state
            h_t = P.tile([128, N_PAD], BF16, name="h_t")
            m_all = P.tile([128, N_PAD], TDT, name="m_all")
            aggT = P.tile([128, N_PAD], BF16, name="aggT")

            # warmup barrier: a tiny collective with no data deps issues at
            # kernel start, absorbing communicator bootstrap cost/skew before
            # the first real AllGather.
            d_warm_in = DR.tile([1, 2], F32, name="warm_in")
            d_warm_out = DR.tile([8, 2], F32, addr_space="Shared",
                                 name="warm_out")
            warm_t = P.tile([1, 2], F32, name="warm_t")
            nc.vector.memset(warm_t[:], 0.0)
            nc.sync.dma_start(out=d_warm_in[:, :], in_=warm_t[:])
            nc.gpsimd.collective_compute(
                "AllGather", mybir.AluOpType.bypass,
                ins=[d_warm_in.opt()], outs=[d_warm_out.opt()],
                replica_groups=[list(range(N_CORES))],
            )

            def msg_tile(t, step):
                pm = PS_M.tile([128, 128], F32, name="pm", tag="pm")
                nc.tensor.matmul(pm[:], lhsT=h_t[:, t * 128:(t + 1) * 128],
                                 rhs=wmsg_b[step % STEPS][:, :],
                                 start=True, stop=True)
                nc.scalar.activation(m_all[:, t * 128:(t + 1) * 128],
                                     pm[:], AF.Copy)

            def send_seg(step, q):
                """DMA m_all segment q to its shard and AllGather it.

                The shard write is split at 512-node chunk boundaries so each
                sub-DMA fires as soon as its chunk's messages are done - the
                collective then only waits on the last small piece."""
                lay = lay_of(step)
                shard = shard_bufs[step][q]
                n0, nn = lay["start"][q], lay["nodes"][q]
                cuts = [n0] + [b for b in range((n0 // 512 + 1) * 512,
                                                n0 + nn, 512)] + [n0 + nn]
                for lo, hi in zip(cuts[:-1], cuts[1:]):
                    src = m_all[:, lo:hi]
                    if ROW_N == 2:
                        # pair-row layout: row r = nodes (2r, 2r+1); node
                        # n=a*128+p lands at row a*64+p//2, offset (p%2)*128
                        out_ap = shard[(lo - n0) // 2:(hi - n0) // 2,
                                       :].rearrange(
                            "(a i) (e b) -> (i e) a b", i=64, e=2)
                    else:
                        out_ap = shard[lo - n0:hi - n0, :].rearrange(
                            "(a p) b -> p a b", p=128)
                    nc.sync.dma_start(
                        out=out_ap,
                        in_=src.rearrange("p (a b) -> p a b", b=128))
                if not DBG_NO_AG:
                    nc.gpsimd.collective_compute(
                        "AllGather", mybir.AluOpType.bypass,
                        ins=[shard.opt()],
                        outs=[table_bufs[step][q].opt()],
                        replica_groups=[list(range(N_CORES))],
                    )

            def ag_after_chunk_for(step):
                """chunk idx after which msg tiles for AG piece q are done"""
                lay = lay_of(step)
                m = {}
                for q in range(lay["n_ag"]):
                    last_tile = (lay["start"][q] + lay["nodes"][q]) // 128 - 1
                    m.setdefault(last_tile // 4, []).append(q)
                return m

            def gru_chunk(ch, step):
                off = ch * 512
                size = min(512, N_PAD - off)
                sl = slice(off, off + size)
                p_r = PS_GRU.tile([128, size], F32, name="p_r", tag="pgru")
                nc.tensor.matmul(p_r[:], lhsT=wih_b[:, 0:128],
                                 rhs=aggT[:, sl], start=True, stop=False)
                nc.tensor.matmul(p_r[:], lhsT=whh_b[:, 0:128],
                                 rhs=h_t[:, sl], start=False, stop=True)
                p_z = PS_GRU.tile([128, size], F32, name="p_z", tag="pgru")
                nc.tensor.matmul(p_z[:], lhsT=wih_b[:, 128:256],
                                 rhs=aggT[:, sl], start=True, stop=False)
                nc.tensor.matmul(p_z[:], lhsT=whh_b[:, 128:256],
                                 rhs=h_t[:, sl], start=False, stop=True)
                p_xn = PS_GRU.tile([128, size], F32, name="p_xn", tag="pgru")
                nc.tensor.matmul(p_xn[:], lhsT=wih_b[:, 256:384],
                                 rhs=aggT[:, sl], start=True, stop=True)
                p_hn = PS_GRU.tile([128, size], F32, name="p_hn", tag="pgru")
                nc.tensor.matmul(p_hn[:], lhsT=whh_b[:, 256:384],
                                 rhs=h_t[:, sl], start=True, stop=True)
                r_t = GP.tile([128, size], BF16, name="r_t", tag="gp1")
                nc.scalar.activation(r_t[:], p_r[:], AF.Sigmoid,
                                     bias=bsum[:, 0:1])
                z_t = GP.tile([128, size], BF16, name="z_t", tag="gp2")
                nc.scalar.activation(z_t[:], p_z[:], AF.Sigmoid,
                                     bias=bsum[:, 1:2])
                hn_t = GP.tile([128, size], BF16, name="hn_t", tag="gp3")
                nc.scalar.activation(hn_t[:], p_hn[:], AF.Identity,
                                     bias=bhh[:, 2:3])
                t1 = GP.tile([128, size], BF16, name="t1", tag="gp4")
                nc.vector.tensor_mul(t1[:], r_t[:], hn_t[:])
                u_t = GP.tile([128, size], F32, name="u_t", tag="gp5")
                nc.vector.tensor_add(u_t[:], t1[:], p_xn[:])
                n_t = GP.tile([128, size], F32, name="n_t", tag="gp6")
                nc.scalar.activation(n_t[:], u_t[:], AF.Tanh,
                                     bias=bih[:, 2:3])
                d_t = GP.tile([128, size], F32, name="d_t", tag="gp7")
                nc.vector.tensor_sub(d_t[:], h_t[:, sl], n_t[:])
                e_t = GP.tile([128, size], F32, name="e_t", tag="gp8")
                nc.vector.tensor_mul(e_t[:], z_t[:], d_t[:])
                nc.vector.tensor_add(h_t[:, sl], n_t[:], e_t[:])

            pool_acc = P.tile([G_PADG, 128], F32, name="pool_acc")

            def readout_chunk(ch):
                # relu commutes with the transpose, so the PSUM->SBUF copy
                # after the transpose applies it - no separate relu pass
                ts = list(range(ch * 4, min((ch + 1) * 4, NT)))
                pq = PS_M.tile([G_PADG, 128], F32, name="pq", tag="pq",
                               bufs=1)
                for i, t in enumerate(ts):
                    ptr2 = PS_M.tile([128, 128], BF16, name="ptr2", tag="pm")
                    nc.tensor.transpose(ptr2[:],
                                        h_t[:, t * 128:(t + 1) * 128],
                                        ident_b[:])
                    hnm = GP.tile([128, 128], BF16, name="hnm", tag="gp1")
                    nc.scalar.activation(hnm[:], ptr2[:], AF.Relu)
                    nc.tensor.matmul(
                        pq[:], lhsT=gmat_b[:, t * G_PADG:(t + 1) * G_PADG],
                        rhs=hnm[:], start=(i == 0), stop=(i == len(ts) - 1))
                if ch == 0:
                    nc.scalar.activation(pool_acc[:], pq[:], AF.Copy)
                else:
                    nc.vector.tensor_add(pool_acc[:], pool_acc[:], pq[:])

            def embed_chunk(ch):
                off = ch * 512
                size = min(512, N_PAD - off)
                pe = PS_GRU.tile([128, size], F32, name="pe_emb", tag="pgru")
                nc.tensor.matmul(pe[:], lhsT=wemb_b[:, :],
                                 rhs=xT_b[:, off:off + size],
                                 start=True, stop=True)
                nc.scalar.activation(h_t[:, off:off + size], pe[:], AF.Relu)

            # deferred loads: needed only after the first AllGather lands
            slot_tiles, ix_tiles = [], []
            for li, el in enumerate(els):
                st_t = P.tile([128, el["t_tot"]], SLOT_DT, name=f"slot_a{li}")
                nc.sync.dma_start(out=st_t[:], in_=d_slot[li][:, :])
                slot_tiles.append(st_t)
                ix_t = P.tile([128, el["t_tot"] * 8], I16, name=f"ix_a{li}")
                nc.sync.dma_start(out=ix_t[:], in_=d_idx[li][:, :])
                ix_tiles.append(ix_t)
            wih_b = load(d_wih[:, :], [H, 3 * H], "wih")
            whh_b = load(d_whh[:, :], [H, 3 * H], "whh")
            bih = load(d_bihT[:, :], [H, 3], "bih", F32)
            bhh = load(d_bhhT[:, :], [H, 3], "bhh", F32)
            bsum = P.tile([H, 3], F32, name="bsum")
            nc.vector.tensor_add(bsum[:], bih[:], bhh[:])
            w1_b = load(d_w1[:, :], [H, H], "w1")
            w2_b = load(d_w2[:, :], [H, 1], "w2")
            b1t = load(d_b1[:, :], [H, 1], "b1t", F32)
            b2t = load(d_b2[:, :], [1, 1], "b2t", F32)
            invc_t = load(d_invc[:, :], [G_PADG, 1], "invc_t", F32)
            gmat_b = load(d_gmat[:, :], [128, NT * G_PADG], "gmat")

            # ---------------- message-passing steps ----------------
            if DBG_STEPS > 0:
                lay0 = lay_of(0)
                # embed + message piece-by-piece so AllGather piece q is not
                # queued behind later chunks' embedding work
                emb_done = 0
                for q in range(lay0["n_ag"]):
                    t0q = lay0["start"][q] // 128
                    need_ch = (t0q + lay0["tiles"][q] + 3) // 4
                    while emb_done < min(need_ch, N_CHUNKS):
                        embed_chunk(emb_done)
                        emb_done += 1
                    for t in range(t0q, t0q + lay0["tiles"][q]):
                        msg_tile(t, 0)
                    send_seg(0, q)
                while emb_done < N_CHUNKS:
                    embed_chunk(emb_done)
                    emb_done += 1
            else:
                for ch in range(N_CHUNKS):
                    embed_chunk(ch)

            for step in range(DBG_STEPS):
                lay = lay_of(step)
                el = els[STEP_LAYOUT[min(step, STEPS - 1)]]
                budget, tb = el["budget"], el["tb"]
                chunk_t0, chunk_nt = el["chunk_t0"], el["chunk_nt"]
                li = STEP_LAYOUT[min(step, STEPS - 1)]
                ix_all, slot_all = ix_tiles[li], slot_tiles[li]
                tab_half = [table_bufs[step][q][:, :]
                            for q in range(lay["n_ag"])]
                ag_after_chunk = (ag_after_chunk_for(step + 1)
                                  if step + 1 < DBG_STEPS else {})

                if DBG_NO_AGG:
                    nc.vector.memset(aggT[:], 0.0)
                for s in range(lay["n_ag"]):
                    last = s == lay["n_ag"] - 1
                    for ch in range(N_CHUNKS):
                        if not DBG_NO_AGG:
                            t0 = int(chunk_t0[ch, s])
                            n_ch = int(chunk_nt[ch, s])
                            if n_ch > 0:
                                E = EP.tile([128, n_ch, ROW_W], TDT,
                                            name="E", tag="E")
                                nc.gpsimd.dma_gather(
                                    E[:], tab_half[s],
                                    ix_all[:, t0 * 8:(t0 + n_ch) * 8],
                                    n_ch * 128, n_ch * 128, ROW_W,
                                    single_packet=bool(int(
                                        os.environ.get("K_SP", "0"))))
                            for g in _chunk_groups(ch):
                                tl = [(int(tb[g, s, p]) + kk, p)
                                      for p in range(ROW_N)
                                      for kk in range(int(budget[g, s, p]))]
                                if not tl:
                                    continue
                                pa = PS_AGG.tile([128, 128], F32, name="pa",
                                                 tag="pa")
                                # one is_equal builds the one-hot St for the
                                # whole cell (tiles are consecutive, so slot
                                # columns broadcast with stride-0 inner dim)
                                nb = len(tl)
                                tg0 = tl[0][0]
                                St_c = SP.tile([128, nb, 128], TDT,
                                               name="St", tag="St")
                                io = (iota_f if TAB_DT_F32 else iota_b)[:]
                                i_rep = bass.AP(
                                    io.tensor, io.offset,
                                    [tuple(io.ap[0]), (0, nb),
                                     tuple(io.ap[1])])
                                sl0 = slot_all[:, tg0:tg0 + 1]
                                s_rep = bass.AP(
                                    sl0.tensor, sl0.offset,
                                    [tuple(sl0.ap[0]), (1, nb), (0, 128)])
                                nc.vector.tensor_tensor(
                                    St_c[:], i_rep, s_rep,
                                    mybir.AluOpType.is_equal)
                                # DoubleRow fp8: two edge tiles per matmul
                                # (256-deep contraction) to halve PE SEQ work
                                if ROW_N == 2:
                                    pairs = [tl[j:j + 2]
                                             for j in range(0, len(tl), 2)]
                                else:
                                    pairs = [tl[j:j + 1]
                                             for j in range(len(tl))]
                                for j, pr in enumerate(pairs):
                                    st_fl = (j == 0, j == len(pairs) - 1)
                                    if len(pr) == 2:
                                        (ta, pa_), (tb_, pb_) = pr
                                        a0 = E[:, ta - t0,
                                               pa_ * 128:pa_ * 128 + 128]
                                        delta = ((tb_ - ta) * ROW_W
                                                 + (pb_ - pa_) * 128)
                                        lhsT2 = bass.AP(
                                            a0.tensor, a0.offset,
                                            [tuple(a0.ap[0]), (delta, 2),
                                             tuple(a0.ap[1])])
                                        nc.tensor.matmul(
                                            pa[:], lhsT=lhsT2,
                                            rhs=St_c[:, 2 * j:2 * j + 2, :],
                                            perf_mode=(mybir.MatmulPerfMode
                                                       .DoubleRow),
                                            start=st_fl[0], stop=st_fl[1])
                                    else:
                                        tg, p = pr[0]
                                        nc.tensor.matmul(
                                            pa[:],
                                            lhsT=E[:, tg - t0,
                                                   p * 128:(p + 1) * 128],
                                            rhs=St_c[:, tg - tg0, :],
                                            start=st_fl[0],
                                            stop=st_fl[1])
                                sl = slice(g * 128, (g + 1) * 128)
                                if s == 0:
                                    nc.scalar.activation(aggT[:, sl], pa[:],
                                                         AF.Copy)
                                else:
                                    nc.vector.tensor_add(aggT[:, sl],
                                                         aggT[:, sl], pa[:])
                        if last:
                            if not DBG_NO_GRU:
                                gru_chunk(ch, step)
                            if step + 1 < DBG_STEPS:
                                for t in range(ch * 4,
                                               min((ch + 1) * 4, NT)):
                                    msg_tile(t, step + 1)
                                for q in ag_after_chunk.get(ch, []):
                                    send_seg(step + 1, q)
                            else:
                                # final step: fold the readout (relu +
                                # transpose + pool matmul) into the chunk
                                # loop so the tail doesn't serialize
                                readout_chunk(ch)

            # ---------------- readout ----------------
            if DBG_STEPS == 0:
                for ch in range(N_CHUNKS):
                    readout_chunk(ch)
            # cross-core pool reduction: AllGather + local sum is cheaper
            # than AllReduce (no 1.875x collective penalty); bf16 partials
            # halve the collective bytes (pooled means tolerate the rounding)
            pool_bf = P.tile([G_PADG, 128], BF16, name="pool_bf")
            nc.vector.tensor_copy(pool_bf[:], pool_acc[:])
            d_pool_in = DR.tile([G_PADG, H], BF16, name="pool_in")
            d_pool_out = DR.tile([N_CORES * G_PADG, H], BF16,
                                 addr_space="Shared", name="pool_out")
            nc.sync.dma_start(out=d_pool_in[:, :], in_=pool_bf[:])
            nc.gpsimd.collective_compute(
                "AllGather", mybir.AluOpType.bypass,
                ins=[d_pool_in.opt()], outs=[d_pool_out.opt()],
                replica_groups=[list(range(N_CORES))],
            )
            pr8 = P.tile([G_PADG, N_CORES, 128], BF16, name="pr8")
            nc.sync.dma_start(
                out=pr8[:],
                in_=d_pool_out.rearrange("(r g) f -> g r f", g=G_PADG))
            pool_r = P.tile([G_PADG, 128], F32, name="pool_r")
            nc.vector.tensor_add(pool_r[:], pr8[:, 0, :], pr8[:, 1, :])
            for r in range(2, N_CORES):
                nc.vector.tensor_add(pool_r[:], pool_r[:], pr8[:, r, :])
            pooled = P.tile([G_PADG, 128], BF16, name="pooled")
            nc.vector.tensor_scalar(pooled[:], pool_r[:], invc_t[:], None,
                                    mybir.AluOpType.mult)
            ppt = PS_M.tile([128, G_PADG], BF16, name="ppt", tag="pm")
            nc.tensor.transpose(ppt[:], pooled[:],
                                ident_b[0:G_PADG, 0:G_PADG])
            pooledT = P.tile([128, G_PADG], BF16, name="pooledT")
            nc.scalar.activation(pooledT[:], ppt[:], AF.Copy)
            pz1 = PS_M.tile([128, G_PADG], F32, name="pz1", tag="pm")
            nc.tensor.matmul(pz1[:], lhsT=w1_b[:, :], rhs=pooledT[:],
                             start=True, stop=True)
            z1 = P.tile([128, G_PADG], BF16, name="z1")
            nc.scalar.activation(z1[:], pz1[:], AF.Relu, bias=b1t[:, 0:1])
            po = PS_M.tile([1, G_PADG], F32, name="po", tag="pm")
            nc.tensor.matmul(po[:], lhsT=w2_b[:, :], rhs=z1[:],
                             start=True, stop=True)
            esb = P.tile([1, G_PADG], F32, name="esb")
            nc.scalar.activation(esb[:], po[:], AF.Exp, bias=b2t[:, 0:1])
            osb = P.tile([1, G_PADG], F32, name="osb")
            nc.scalar.activation(osb[:], esb[:], AF.Ln, bias=1.0)
            nc.sync.dma_start(out=d_out[:, :], in_=osb[:])

    nc.compile()
    return nc


# ----------------------------------------------------------------------------
# entry point
# ----------------------------------------------------------------------------

def make_in_maps(inputs, per_core, meta):
    return _make_in_maps(per_core, meta, **{
        k: inputs[k] for k in ("W_emb", "W_msg", "W_ih", "W_hh", "b_ih",
                               "b_hh", "W1", "b1", "W2", "b2")})


def _make_in_maps(per_core, meta, W_emb, W_msg, W_ih, W_hh, b_ih, b_hh,
                  W1, b1, W2, b2):
    bf = ml_dtypes.bfloat16
    shared = dict(
        iota=np.arange(128, dtype=np.float32).reshape(1, 128),
        ident=np.eye(128, dtype=np.float32).astype(bf),
        wemb=np.asarray(W_emb, np.float32).astype(bf),
        wmsg=np.asarray(W_msg, np.float32).astype(bf),
        wih=np.asarray(W_ih, np.float32).astype(bf),
        whh=np.asarray(W_hh, np.float32).astype(bf),
        bihT=np.ascontiguousarray(
            np.asarray(b_ih, np.float32).reshape(3, H).T),
        bhhT=np.ascontiguousarray(
            np.asarray(b_hh, np.float32).reshape(3, H).T),
        w1=np.asarray(W1, np.float32).astype(bf),
        b1=np.asarray(b1, np.float32).reshape(H, 1),
        w2=np.asarray(W2, np.float32).astype(bf),
        b2=np.asarray(b2, np.float32).reshape(1, 1),
        invc=meta["invc"],
    )
    in_maps = []
    for c in range(N_CORES):
        m = dict(shared)
        m["xT"] = per_core[c]["xT"]
        m["gmat"] = per_core[c]["gmat"]
        for li in range(len(LAYOUTS)):
            m[f"idx{li}"] = per_core[c][f"idx{li}"]
            m[f"slot{li}"] = per_core[c][f"slot{li}"]
        in_maps.append(m)
    return in_maps


def kernel(x, edge_index, batch, W_emb, W_msg, W_ih, W_hh, b_ih, b_hh,
           W1, b1, W2, b2):
    per_core, meta = _preprocess(x, edge_index, batch)
    nc = _build(meta)
    in_maps = _make_in_maps(per_core, meta, W_emb, W_msg, W_ih, W_hh,
                            b_ih, b_hh, W1, b1, W2, b2)

    trace = bool(int(os.environ.get("KERNEL_TRACE", "0")))
    res = run_bass_kernel_spmd(nc, in_maps, list(range(N_CORES)), trace=trace)
    LAST_RESULTS["exec_time_ns"] = res.exec_time_ns
    LAST_RESULTS["profile_json"] = res.profile_json
    LAST_RESULTS["nc"] = nc
    LAST_RESULTS["in_maps"] = in_maps

    return np.asarray(res.results[0]["out"][0, :N_GRAPHS], np.float32)



# revision 75
# speedup vs baseline: 1.0293x; 1.0114x over previous
"""Trainium2 Bass kernel for BondingGraphGNN (gnn_message_passing), v3.

Model (see reference):
  h = relu(x @ W_emb)
  4x: m = h @ W_msg[i]; agg = scatter_add(m[src] -> dst); h = GRU(agg, h)
  h = relu(h); pooled = segment_mean(h, batch); out = softplus(relu(pooled@W1+b1)@W2+b2)

Distribution: even node sharding (6250 nodes/core, padded). Per step each core
computes messages for its nodes, AllGathers the message table to DRAM, and
aggregates its incoming edges locally with a gather + one-hot-matmul
segment-sum, then runs the GRU.

v3 performance structure (~2.3x over v2 in the cost model):
- fp8e4 message table in pair-rows ([2 nodes, 256B] per row, parity-pure edge
  tiles pick their half via the lhsT offset): halves AllGather bytes; the
  scatter-sum averages ~16 messages so fp8 noise washes out (rel err ~1e-3).
- DoubleRow fp8 matmuls: two 128-edge tiles (256-deep contraction) per PE
  instruction via a custom strided lhsT AP - halves PE sequencer work, which
  otherwise bounds the aggregation.
- One DVE is_equal per (group, segment) cell builds the whole one-hot St
  stack (consecutive slot columns, stride-0 broadcast APs).
- Per-step AllGather in 3 pieces [16,17,17] tiles, consumed pass-by-pass
  (pass 0 initializes aggT, later passes accumulate). The small piece 0
  means the next step's first AllGather only waits on 4 GRU chunks, and the
  last pass's gathers hide under its flight - the collective engine streams
  nearly back-to-back.
- Step-invariant gather indices/slots preloaded once; startup reordered so
  the embedding's xT DMA precedes bulk prefetches; shard writes split at
  chunk boundaries so each fires as messages complete; readout folded into
  the last step's chunk loop; pool reduction via AllGather + local sum
  (cheaper than AllReduce); a tiny warmup AllGather absorbs communicator
  bootstrap before the first real collective.

Readout: per-core partial pooling + AllGather + local sum + tiny MLP
replicated on every core (host takes core 0).
"""

import os
import numpy as np

# the trimmed axon package in some containers lacks the NTFF profile hook
# module; stub it so run_bass_kernel_spmd(trace=True) degrades gracefully.
import sys as _sys, types as _types
try:
    import antenv.axon_hooks  # noqa: F401
except Exception:
    _m = _types.ModuleType("antenv.axon_hooks")
    _m.get_axon_ntff_profile_hook = lambda: None
    _sys.modules["antenv.axon_hooks"] = _m

import ml_dtypes
import concourse.bacc as bacc
import concourse.bass as bass
import concourse.mybir as mybir
import concourse.tile as tile
from concourse.bass_utils import run_bass_kernel_spmd

F32 = mybir.dt.float32
BF16 = mybir.dt.bfloat16
F8 = mybir.dt.float8e4
I16 = mybir.dt.int16
AF = mybir.ActivationFunctionType

N_NODES = 50000
N_EDGES = 800000
FEAT = 90
H = 128
STEPS = 4
N_GRAPHS = 100
N_CORES = 8

NC_NODES = N_NODES // N_CORES               # 6250 real nodes per core
N_PAD = 6400
NT = N_PAD // 128                           # dst groups per core (50)
N_CHUNKS = (NT + 3) // 4                    # 13 chunks of <=4 groups
G_PADG = 112                                # padded global graph count
PAD_SLOT = 255.0                            # sentinel slot -> all-zero S row

_TAB_NAME = os.environ.get("K_TAB", "f8")   # f32 | bf16 | f8
ROW_N = 2 if _TAB_NAME == "f8" else 1       # table nodes per row (fp8 pairs)
ROW_W = ROW_N * 128                         # table row width, elements


def _mk_layout(seg_tiles):
    """Per-step AllGather piece layout. Small first piece: the next step's
    first AllGather only waits on a few GRU chunks. The pieces need to cover
    only the REAL sources (0..NC_NODES-1); trailing pad tiles are neither
    messaged nor gathered."""
    assert NC_NODES <= sum(seg_tiles) * 128 <= N_PAD
    nodes = [t * 128 for t in seg_tiles]
    start = [sum(nodes[:q]) for q in range(len(seg_tiles))]
    rows = [N_CORES * n for n in nodes]
    assert max(rows) // ROW_N <= 32768
    return dict(tiles=seg_tiles, nodes=nodes, start=start, rows=rows,
                n_ag=len(seg_tiles))


def _parse_segs(env, default):
    v = os.environ.get(env, "")
    return [int(x) for x in v.split(",")] if v else default


# per-step piece layouts (a distinct last-step layout is supported via
# K_SEGS_LAST but the uniform split benches best)
LAYOUTS = [_mk_layout(_parse_segs("K_SEGS", [16, 17, 16])),
           _mk_layout(_parse_segs("K_SEGS_LAST", [16, 17, 16]))]
STEP_LAYOUT = [0, 0, 0, 1]
if LAYOUTS[1]["tiles"] == LAYOUTS[0]["tiles"]:
    LAYOUTS = LAYOUTS[:1]
    STEP_LAYOUT = [0, 0, 0, 0]

TAB_DT_F32 = _TAB_NAME == "f32"
LAST_RESULTS = {}   # stash for test.py (exec time etc)


def _chunk_groups(ch):
    return range(ch * 4, min((ch + 1) * 4, NT))


# ----------------------------------------------------------------------------
# host-side layout
# ----------------------------------------------------------------------------

def _edge_layout(lay, d_core, grp, slot, s_core, s_local):
    """Tile/slot/idx tables for one AllGather piece layout."""
    n_seg = lay["n_ag"]
    bounds = np.asarray(lay["start"][1:]
                        + [lay["start"][-1] + lay["nodes"][-1]], np.int64)
    seg = np.searchsorted(bounds, s_local, side="right")
    assert seg.max() < n_seg, "pieces must cover all real sources"
    seg_nodes = np.asarray(lay["nodes"], np.int64)[seg]
    seg_start = np.asarray(lay["start"], np.int64)[seg]
    loc = s_core * seg_nodes + (s_local - seg_start)
    if ROW_N == 2:
        # fp8 pair-row table: row r of a piece holds sources (2r, 2r+1);
        # each tile is parity-pure so the matmul lhsT picks one half.
        par = loc % 2
        idxval = loc // 2
    else:
        par = np.zeros_like(loc)
        idxval = loc

    # per (core, grp, seg, par) counts -> uniform budgets
    cnt = np.zeros((N_CORES, NT, n_seg, ROW_N), np.int64)
    np.add.at(cnt, (d_core, grp, seg, par), 1)
    budget = np.ceil(cnt.max(axis=0) / 128).astype(np.int64)
    # every group needs >=1 pass-0 tile so the copy flush writes aggT
    need = budget[:, 0, :].sum(axis=1) == 0
    budget[need, 0, 0] = 1

    # tile order: seg-major, then chunk, then group, then parity
    tb = np.zeros((NT, n_seg, ROW_N), np.int64)
    chunk_t0 = np.zeros((N_CHUNKS, n_seg), np.int64)
    chunk_nt = np.zeros((N_CHUNKS, n_seg), np.int64)
    t = 0
    for s in range(n_seg):
        for ch in range(N_CHUNKS):
            chunk_t0[ch, s] = t
            for g in _chunk_groups(ch):
                for p in range(ROW_N):
                    tb[g, s, p] = t
                    t += int(budget[g, s, p])
            chunk_nt[ch, s] = t - chunk_t0[ch, s]
    t_tot = t

    # edge placement (vectorized)
    order = np.lexsort((par, grp, seg, d_core))
    sc = d_core[order]
    ss = seg[order]
    sg = grp[order]
    sp = par[order]
    sidx = idxval[order]
    sslot = slot[order]
    rid = ((sc * n_seg + ss) * NT + sg) * ROW_N + sp
    run_first = np.r_[0, np.flatnonzero(np.diff(rid)) + 1]
    run_len = np.diff(np.r_[run_first, len(rid)])
    k = np.arange(len(rid)) - np.repeat(run_first, run_len)
    tt = tb[sg, ss, sp] + k // 128
    pp = k % 128
    ct0 = chunk_t0[sg // 4, ss]
    pos = (tt - ct0) * 128 + pp

    idx_arr = np.zeros((N_CORES, 16, t_tot * 8), np.int16)
    slot_arr = np.full((N_CORES, 128, t_tot), PAD_SLOT, np.float32)
    idx_arr[sc, pos % 16, ct0 * 8 + pos // 16] = sidx.astype(np.int16)
    slot_arr[sc, pp, tt] = sslot
    return dict(budget=budget, tb=tb, chunk_t0=chunk_t0, chunk_nt=chunk_nt,
                t_tot=t_tot, idx_arr=idx_arr, slot_arr=slot_arr)


def _preprocess(x, edge_index, batch):
    batch = np.asarray(batch, np.int64)
    src = np.asarray(edge_index[0], np.int64)
    dst = np.asarray(edge_index[1], np.int64)
    frac = float(os.environ.get("K_EDGE_FRAC", "1"))
    if frac < 1.0:  # timing experiments only - wrong results
        n = int(len(src) * frac)
        src, dst = src[:n], dst[:n]

    d_core = dst // NC_NODES
    d_local = dst - d_core * NC_NODES
    grp = d_local // 128
    slot = (d_local % 128).astype(np.float32)
    s_core = src // NC_NODES
    s_local = src - s_core * NC_NODES

    els = [_edge_layout(lay, d_core, grp, slot, s_core, s_local)
           for lay in LAYOUTS]

    # per-core node features (transposed, padded, bf16) and graph one-hots
    counts = np.bincount(batch, minlength=N_GRAPHS).astype(np.float32)
    invc = np.zeros((G_PADG, 1), np.float32)
    invc[:N_GRAPHS, 0] = 1.0 / np.maximum(counts, 1.0)
    x = np.asarray(x, np.float32)
    slot_dt = np.float32 if TAB_DT_F32 else ml_dtypes.bfloat16
    per_core = []
    for c in range(N_CORES):
        n0 = c * NC_NODES
        xT = np.zeros((FEAT, N_PAD), np.float32)
        xT[:, :NC_NODES] = x[n0:n0 + NC_NODES].T
        gmat = np.zeros((128, NT * G_PADG), np.float32)
        l = np.arange(NC_NODES)
        gmat[l % 128, (l // 128) * G_PADG + batch[n0:n0 + NC_NODES]] = 1.0
        pc = dict(
            xT=xT.astype(ml_dtypes.bfloat16),
            gmat=gmat.astype(ml_dtypes.bfloat16),
        )
        for li, el in enumerate(els):
            pc[f"idx{li}"] = np.tile(el["idx_arr"][c], (8, 1))
            pc[f"slot{li}"] = el["slot_arr"][c].astype(slot_dt)
        per_core.append(pc)

    meta = dict(els=els, invc=invc)
    return per_core, meta


# ----------------------------------------------------------------------------
# device program
# ----------------------------------------------------------------------------

def _build(meta):
    DBG_STEPS = int(os.environ.get("K_STEPS", STEPS))
    DBG_NO_AG = bool(int(os.environ.get("K_NO_AG", "0")))
    DBG_NO_AGG = bool(int(os.environ.get("K_NO_AGG", "0")))
    DBG_NO_GRU = bool(int(os.environ.get("K_NO_GRU", "0")))
    els = meta["els"]

    nc = bacc.Bacc("TRN2", target_bir_lowering=False, debug=False,
                   num_devices=N_CORES)

    d_xT = nc.dram_tensor("xT", [FEAT, N_PAD], BF16, kind="ExternalInput")
    SLOT_DT = F32 if TAB_DT_F32 else BF16
    d_idx = [nc.dram_tensor(f"idx{li}", [128, el["t_tot"] * 8], I16,
                            kind="ExternalInput")
             for li, el in enumerate(els)]
    d_slot = [nc.dram_tensor(f"slot{li}", [128, el["t_tot"]], SLOT_DT,
                             kind="ExternalInput")
              for li, el in enumerate(els)]
    d_gmat = nc.dram_tensor("gmat", [128, NT * G_PADG], BF16,
                            kind="ExternalInput")
    d_invc = nc.dram_tensor("invc", [G_PADG, 1], F32, kind="ExternalInput")
    d_iota = nc.dram_tensor("iota", [1, 128], F32, kind="ExternalInput")
    d_ident = nc.dram_tensor("ident", [128, 128], BF16, kind="ExternalInput")
    d_wemb = nc.dram_tensor("wemb", [FEAT, H], BF16, kind="ExternalInput")
    d_wmsg = nc.dram_tensor("wmsg", [STEPS, H, H], BF16, kind="ExternalInput")
    d_wih = nc.dram_tensor("wih", [H, 3 * H], BF16, kind="ExternalInput")
    d_whh = nc.dram_tensor("whh", [H, 3 * H], BF16, kind="ExternalInput")
    d_bihT = nc.dram_tensor("bihT", [H, 3], F32, kind="ExternalInput")
    d_bhhT = nc.dram_tensor("bhhT", [H, 3], F32, kind="ExternalInput")
    d_w1 = nc.dram_tensor("w1", [H, H], BF16, kind="ExternalInput")
    d_b1 = nc.dram_tensor("b1", [H, 1], F32, kind="ExternalInput")
    d_w2 = nc.dram_tensor("w2", [H, 1], BF16, kind="ExternalInput")
    d_b2 = nc.dram_tensor("b2", [1, 1], F32, kind="ExternalInput")
    d_out = nc.dram_tensor("out", [1, G_PADG], F32, kind="ExternalOutput")

    with tile.TileContext(nc) as tc:
        with (
            tc.tile_pool(name="persist", bufs=1) as P,
            tc.tile_pool(name="dram", bufs=1, space="DRAM") as DR,
            tc.tile_pool(name="epool",
                         bufs=int(os.environ.get("K_EP", "4"))) as EP,
            tc.tile_pool(name="spool",
                         bufs=int(os.environ.get("K_SPOOL", "8"))) as SP,
            tc.tile_pool(name="gpool", bufs=2) as GP,
            tc.tile_pool(name="ps_agg", space="PSUM",
                         bufs=int(os.environ.get("K_PA", "2"))) as PS_AGG,
            tc.tile_pool(name="ps_m", bufs=1, space="PSUM") as PS_M,
            tc.tile_pool(name="ps_gru", space="PSUM",
                         bufs=int(os.environ.get("K_PG", "4"))) as PS_GRU,
        ):
            # DRAM temps: per-step message shards and gathered tables
            TDT = {"f32": F32, "bf16": BF16, "f8": F8}[_TAB_NAME]
            n_tab = max(DBG_STEPS, 1)

            def lay_of(step):
                return LAYOUTS[STEP_LAYOUT[min(step, STEPS - 1)]]

            shard_bufs = [[DR.tile([lay_of(st)["nodes"][q] // ROW_N, ROW_W],
                                   TDT, name=f"m_shard{st}_{q}")
                           for q in range(lay_of(st)["n_ag"])]
                          for st in range(n_tab)]
            table_bufs = [[DR.tile([lay_of(st)["rows"][q] // ROW_N, ROW_W],
                                   TDT, addr_space="Shared",
                                   name=f"m_table{st}_{q}")
                           for q in range(lay_of(st)["n_ag"])]
                          for st in range(n_tab)]

            # ------- startup-critical loads first (xT feeds embedding) -----
            def load(dram_ap, shape, name, dt=BF16):
                tl = P.tile(shape, dt, name=name)
                nc.sync.dma_start(out=tl[:], in_=dram_ap)
                return tl

            xT_b = P.tile([FEAT, N_PAD], BF16, name="xT_b")
            nc.sync.dma_start(out=xT_b[:], in_=d_xT[:, :])
            wemb_b = load(d_wemb[:, :], [FEAT, H], "wemb")
            wmsg_b = [load(d_wmsg[s, :, :], [H, H], f"wmsg{s}")
                      for s in range(STEPS)]
            iota_f = P.tile([128, 128], F32, name="iota_f")
            nc.sync.dma_start(out=iota_f[:],
                              in_=d_iota.ap().to_broadcast([128, 128]))
            iota_b = P.tile([128, 128], BF16, name="iota_b")
            nc.vector.tensor_copy(iota_b[:], iota_f[:])
            ident_b = P.tile([128, 128], BF16, name="ident_b")
            nc.sync.dma_start(out=ident_b[:], in_=d_ident[:, :])

            # state
            h_t = P.tile([128, N_PAD], BF16, name="h_t")
            m_all = P.tile([128, N_PAD], TDT, name="m_all")
            aggT = P.tile([128, N_PAD], BF16, name="aggT")

            # warmup barrier: a tiny collective with no data deps issues at
            # kernel start, absorbing communicator bootstrap cost/skew before
            # the first real AllGather.
            d_warm_in = DR.tile([1, 2], F32, name="warm_in")
            d_warm_out = DR.tile([8, 2], F32, addr_space="Shared",
                                 name="warm_out")
            warm_t = P.tile([1, 2], F32, name="warm_t")
            nc.vector.memset(warm_t[:], 0.0)
            nc.sync.dma_start(out=d_warm_in[:, :], in_=warm_t[:])
            nc.gpsimd.collective_compute(
                "AllGather", mybir.AluOpType.bypass,
                ins=[d_warm_in.opt()], outs=[d_warm_out.opt()],
                replica_groups=[list(range(N_CORES))],
            )

            def msg_tile(t, step):
                pm = PS_M.tile([128, 128], F32, name="pm", tag="pm")
                nc.tensor.matmul(pm[:], lhsT=h_t[:, t * 128:(t + 1) * 128],
                                 rhs=wmsg_b[step % STEPS][:, :],
                                 start=True, stop=True)
                nc.scalar.activation(m_all[:, t * 128:(t + 1) * 128],
                                     pm[:], AF.Copy)

            def send_seg(step, q):
                """DMA m_all segment q to its shard and AllGather it.

                The shard write is split at 512-node chunk boundaries so each
                sub-DMA fires as soon as its chunk's messages are done - the
                collective then only waits on the last small piece."""
                lay = lay_of(step)
                shard = shard_bufs[step][q]
                n0, nn = lay["start"][q], lay["nodes"][q]
                cuts = [n0] + [b for b in range((n0 // 512 + 1) * 512,
                                                n0 + nn, 512)] + [n0 + nn]
                for lo, hi in zip(cuts[:-1], cuts[1:]):
                    src = m_all[:, lo:hi]
                    if ROW_N == 2:
                        # pair-row layout: row r = nodes (2r, 2r+1); node
                        # n=a*128+p lands at row a*64+p//2, offset (p%2)*128
                        out_ap = shard[(lo - n0) // 2:(hi - n0) // 2,
                                       :].rearrange(
                            "(a i) (e b) -> (i e) a b", i=64, e=2)
                    else:
                        out_ap = shard[lo - n0:hi - n0, :].rearrange(
                            "(a p) b -> p a b", p=128)
                    nc.sync.dma_start(
                        out=out_ap,
                        in_=src.rearrange("p (a b) -> p a b", b=128))
                if not DBG_NO_AG:
                    nc.gpsimd.collective_compute(
                        "AllGather", mybir.AluOpType.bypass,
                        ins=[shard.opt()],
                        outs=[table_bufs[step][q].opt()],
                        replica_groups=[list(range(N_CORES))],
                    )

            def ag_after_chunk_for(step):
                """chunk idx after which msg tiles for AG piece q are done"""
                lay = lay_of(step)
                m = {}
                for q in range(lay["n_ag"]):
                    last_tile = (lay["start"][q] + lay["nodes"][q]) // 128 - 1
                    m.setdefault(last_tile // 4, []).append(q)
                return m

            def gru_chunk(ch, step):
                off = ch * 512
                size = min(512, N_PAD - off)
                sl = slice(off, off + size)
                p_r = PS_GRU.tile([128, size], F32, name="p_r", tag="pgru")
                nc.tensor.matmul(p_r[:], lhsT=wih_b[:, 0:128],
                                 rhs=aggT[:, sl], start=True, stop=False)
                nc.tensor.matmul(p_r[:], lhsT=whh_b[:, 0:128],
                                 rhs=h_t[:, sl], start=False, stop=True)
                p_z = PS_GRU.tile([128, size], F32, name="p_z", tag="pgru")
                nc.tensor.matmul(p_z[:], lhsT=wih_b[:, 128:256],
                                 rhs=aggT[:, sl], start=True, stop=False)
                nc.tensor.matmul(p_z[:], lhsT=whh_b[:, 128:256],
                                 rhs=h_t[:, sl], start=False, stop=True)
                p_xn = PS_GRU.tile([128, size], F32, name="p_xn", tag="pgru")
                nc.tensor.matmul(p_xn[:], lhsT=wih_b[:, 256:384],
                                 rhs=aggT[:, sl], start=True, stop=True)
                p_hn = PS_GRU.tile([128, size], F32, name="p_hn", tag="pgru")
                nc.tensor.matmul(p_hn[:], lhsT=whh_b[:, 256:384],
                                 rhs=h_t[:, sl], start=True, stop=True)
                r_t = GP.tile([128, size], BF16, name="r_t", tag="gp1")
                nc.scalar.activation(r_t[:], p_r[:], AF.Sigmoid,
                                     bias=bsum[:, 0:1])
                z_t = GP.tile([128, size], BF16, name="z_t", tag="gp2")
                nc.scalar.activation(z_t[:], p_z[:], AF.Sigmoid,
                                     bias=bsum[:, 1:2])
                hn_t = GP.tile([128, size], BF16, name="hn_t", tag="gp3")
                nc.scalar.activation(hn_t[:], p_hn[:], AF.Identity,
                                     bias=bhh[:, 2:3])
                t1 = GP.tile([128, size], BF16, name="t1", tag="gp4")
                nc.vector.tensor_mul(t1[:], r_t[:], hn_t[:])
                u_t = GP.tile([128, size], F32, name="u_t", tag="gp5")
                nc.vector.tensor_add(u_t[:], t1[:], p_xn[:])
                n_t = GP.tile([128, size], F32, name="n_t", tag="gp6")
                nc.scalar.activation(n_t[:], u_t[:], AF.Tanh,
                                     bias=bih[:, 2:3])
                d_t = GP.tile([128, size], F32, name="d_t", tag="gp7")
                nc.vector.tensor_sub(d_t[:], h_t[:, sl], n_t[:])
                e_t = GP.tile([128, size], F32, name="e_t", tag="gp8")
                nc.vector.tensor_mul(e_t[:], z_t[:], d_t[:])
                nc.vector.tensor_add(h_t[:, sl], n_t[:], e_t[:])

            pool_acc = P.tile([G_PADG, 128], F32, name="pool_acc")

            def readout_chunk(ch):
                # relu commutes with the transpose, so the PSUM->SBUF copy
                # after the transpose applies it - no separate relu pass
                ts = list(range(ch * 4, min((ch + 1) * 4, NT)))
                pq = PS_M.tile([G_PADG, 128], F32, name="pq", tag="pq",
                               bufs=1)
                for i, t in enumerate(ts):
                    ptr2 = PS_M.tile([128, 128], BF16, name="ptr2", tag="pm")
                    nc.tensor.transpose(ptr2[:],
                                        h_t[:, t * 128:(t + 1) * 128],
                                        ident_b[:])
                    hnm = GP.tile([128, 128], BF16, name="hnm", tag="gp1")
                    nc.scalar.activation(hnm[:], ptr2[:], AF.Relu)
                    nc.tensor.matmul(
                        pq[:], lhsT=gmat_b[:, t * G_PADG:(t + 1) * G_PADG],
                        rhs=hnm[:], start=(i == 0), stop=(i == len(ts) - 1))
                if ch == 0:
                    nc.scalar.activation(pool_acc[:], pq[:], AF.Copy)
                else:
                    nc.vector.tensor_add(pool_acc[:], pool_acc[:], pq[:])

            def embed_chunk(ch):
                off = ch * 512
                size = min(512, N_PAD - off)
                pe = PS_GRU.tile([128, size], F32, name="pe_emb", tag="pgru")
                nc.tensor.matmul(pe[:], lhsT=wemb_b[:, :],
                                 rhs=xT_b[:, off:off + size],
                                 start=True, stop=True)
                nc.scalar.activation(h_t[:, off:off + size], pe[:], AF.Relu)

            # deferred loads: needed only after the first AllGather lands
            slot_tiles, ix_tiles = [], []
            for li, el in enumerate(els):
                st_t = P.tile([128, el["t_tot"]], SLOT_DT, name=f"slot_a{li}")
                nc.sync.dma_start(out=st_t[:], in_=d_slot[li][:, :])
                slot_tiles.append(st_t)
                ix_t = P.tile([128, el["t_tot"] * 8], I16, name=f"ix_a{li}")
                nc.sync.dma_start(out=ix_t[:], in_=d_idx[li][:, :])
                ix_tiles.append(ix_t)
            wih_b = load(d_wih[:, :], [H, 3 * H], "wih")
            whh_b = load(d_whh[:, :], [H, 3 * H], "whh")
            bih = load(d_bihT[:, :], [H, 3], "bih", F32)
            bhh = load(d_bhhT[:, :], [H, 3], "bhh", F32)
            bsum = P.tile([H, 3], F32, name="bsum")
            nc.vector.tensor_add(bsum[:], bih[:], bhh[:])
            w1_b = load(d_w1[:, :], [H, H], "w1")
            w2_b = load(d_w2[:, :], [H, 1], "w2")
            b1t = load(d_b1[:, :], [H, 1], "b1t", F32)
            b2t = load(d_b2[:, :], [1, 1], "b2t", F32)
            invc_t = load(d_invc[:, :], [G_PADG, 1], "invc_t", F32)
            gmat_b = load(d_gmat[:, :], [128, NT * G_PADG], "gmat")

            # ---------------- message-passing steps ----------------
            if DBG_STEPS > 0:
                lay0 = lay_of(0)
                # embed + message piece-by-piece so AllGather piece q is not
                # queued behind later chunks' embedding work
                emb_done = 0
                for q in range(lay0["n_ag"]):
                    t0q = lay0["start"][q] // 128
                    need_ch = (t0q + lay0["tiles"][q] + 3) // 4
                    while emb_done < min(need_ch, N_CHUNKS):
                        embed_chunk(emb_done)
                        emb_done += 1
                    for t in range(t0q, t0q + lay0["tiles"][q]):
                        msg_tile(t, 0)
                    send_seg(0, q)
                while emb_done < N_CHUNKS:
                    embed_chunk(emb_done)
                    emb_done += 1
            else:
                for ch in range(N_CHUNKS):
                    embed_chunk(ch)

            for step in range(DBG_STEPS):
                lay = lay_of(step)
                el = els[STEP_LAYOUT[min(step, STEPS - 1)]]
                budget, tb = el["budget"], el["tb"]
                chunk_t0, chunk_nt = el["chunk_t0"], el["chunk_nt"]
                li = STEP_LAYOUT[min(step, STEPS - 1)]
                ix_all, slot_all = ix_tiles[li], slot_tiles[li]
                tab_half = [table_bufs[step][q][:, :]
                            for q in range(lay["n_ag"])]
                ag_after_chunk = (ag_after_chunk_for(step + 1)
                                  if step + 1 < DBG_STEPS else {})

                if DBG_NO_AGG:
                    nc.vector.memset(aggT[:], 0.0)
                for s in range(lay["n_ag"]):
                    last = s == lay["n_ag"] - 1
                    for ch in range(N_CHUNKS):
                        if not DBG_NO_AGG:
                            t0 = int(chunk_t0[ch, s])
                            n_ch = int(chunk_nt[ch, s])
                            if n_ch > 0:
                                E = EP.tile([128, n_ch, ROW_W], TDT,
                                            name="E", tag="E")
                                nc.gpsimd.dma_gather(
                                    E[:], tab_half[s],
                                    ix_all[:, t0 * 8:(t0 + n_ch) * 8],
                                    n_ch * 128, n_ch * 128, ROW_W,
                                    single_packet=bool(int(
                                        os.environ.get("K_SP", "0"))))
                            for g in _chunk_groups(ch):
                                tl = [(int(tb[g, s, p]) + kk, p)
                                      for p in range(ROW_N)
                                      for kk in range(int(budget[g, s, p]))]
                                if not tl:
                                    continue
                                pa = PS_AGG.tile([128, 128], F32, name="pa",
                                                 tag="pa")
                                # one is_equal builds the one-hot St for the
                                # whole cell (tiles are consecutive, so slot
                                # columns broadcast with stride-0 inner dim)
                                nb = len(tl)
                                tg0 = tl[0][0]
                                St_c = SP.tile([128, nb, 128], TDT,
                                               name="St", tag="St")
                                io = (iota_f if TAB_DT_F32 else iota_b)[:]
                                i_rep = bass.AP(
                                    io.tensor, io.offset,
                                    [tuple(io.ap[0]), (0, nb),
                                     tuple(io.ap[1])])
                                sl0 = slot_all[:, tg0:tg0 + 1]
                                s_rep = bass.AP(
                                    sl0.tensor, sl0.offset,
                                    [tuple(sl0.ap[0]), (1, nb), (0, 128)])
                                nc.vector.tensor_tensor(
                                    St_c[:], i_rep, s_rep,
                                    mybir.AluOpType.is_equal)
                                # DoubleRow fp8: two edge tiles per matmul
                                # (256-deep contraction) to halve PE SEQ work
                                if ROW_N == 2:
                                    pairs = [tl[j:j + 2]
                                             for j in range(0, len(tl), 2)]
                                else:
                                    pairs = [tl[j:j + 1]
                                             for j in range(len(tl))]
                                for j, pr in enumerate(pairs):
                                    st_fl = (j == 0, j == len(pairs) - 1)
                                    if len(pr) == 2:
                                        (ta, pa_), (tb_, pb_) = pr
                                        a0 = E[:, ta - t0,
                                               pa_ * 128:pa_ * 128 + 128]
                                        delta = ((tb_ - ta) * ROW_W
                                                 + (pb_ - pa_) * 128)
                                        lhsT2 = bass.AP(
                                            a0.tensor, a0.offset,
                                            [tuple(a0.ap[0]), (delta, 2),
                                             tuple(a0.ap[1])])
                                        nc.tensor.matmul(
                                            pa[:], lhsT=lhsT2,
                                            rhs=St_c[:, 2 * j:2 * j + 2, :],
                                            perf_mode=(mybir.MatmulPerfMode
                                                       .DoubleRow),
                                            start=st_fl[0], stop=st_fl[1])
                                    else:
                                        tg, p = pr[0]
                                        nc.tensor.matmul(
                                            pa[:],
                                            lhsT=E[:, tg - t0,
                                                   p * 128:(p + 1) * 128],
                                            rhs=St_c[:, tg - tg0, :],
                                            start=st_fl[0],
                                            stop=st_fl[1])
                                sl = slice(g * 128, (g + 1) * 128)
                                if s == 0:
                                    nc.scalar.activation(aggT[:, sl], pa[:],
                                                         AF.Copy)
                                else:
                                    nc.vector.tensor_add(aggT[:, sl],
                                                         aggT[:, sl], pa[:])
                        if last:
                            if not DBG_NO_GRU:
                                gru_chunk(ch, step)
                            if step + 1 < DBG_STEPS:
                                for t in range(ch * 4,
                                               min((ch + 1) * 4, NT)):
                                    msg_tile(t, step + 1)
                                for q in ag_after_chunk.get(ch, []):
                                    send_seg(step + 1, q)
                            else:
                                # final step: fold the readout (relu +
                                # transpose + pool matmul) into the chunk
                                # loop so the tail doesn't serialize
                                readout_chunk(ch)

            # ---------------- readout ----------------
            if DBG_STEPS == 0:
                for ch in range(N_CHUNKS):
                    readout_chunk(ch)
            # cross-core pool reduction: AllGather + local sum is cheaper
            # than AllReduce (no 1.875x collective penalty); bf16 partials
            # halve the collective bytes (pooled means tolerate the rounding)
            pool_bf = P.tile([G_PADG, 128], BF16, name="pool_bf")
            nc.vector.tensor_copy(pool_bf[:], pool_acc[:])
            d_pool_in = DR.tile([G_PADG, H], BF16, name="pool_in")
            d_pool_out = DR.tile([N_CORES * G_PADG, H], BF16,
                                 addr_space="Shared", name="pool_out")
            nc.sync.dma_start(out=d_pool_in[:, :], in_=pool_bf[:])
            nc.gpsimd.collective_compute(
                "AllGather", mybir.AluOpType.bypass,
                ins=[d_pool_in.opt()], outs=[d_pool_out.opt()],
                replica_groups=[list(range(N_CORES))],
            )
            pr8 = P.tile([G_PADG, N_CORES, 128], BF16, name="pr8")
            nc.sync.dma_start(
                out=pr8[:],
                in_=d_pool_out.rearrange("(r g) f -> g r f", g=G_PADG))
            pool_r = P.tile([G_PADG, 128], F32, name="pool_r")
            nc.vector.tensor_add(pool_r[:], pr8[:, 0, :], pr8[:, 1, :])
            for r in range(2, N_CORES):
                nc.vector.tensor_add(pool_r[:], pool_r[:], pr8[:, r, :])
            pooled = P.tile([G_PADG, 128], BF16, name="pooled")
            nc.vector.tensor_scalar(pooled[:], pool_r[:], invc_t[:], None,
                                    mybir.AluOpType.mult)
            ppt = PS_M.tile([128, G_PADG], BF16, name="ppt", tag="pm")
            nc.tensor.transpose(ppt[:], pooled[:],
                                ident_b[0:G_PADG, 0:G_PADG])
            pooledT = P.tile([128, G_PADG], BF16, name="pooledT")
            nc.scalar.activation(pooledT[:], ppt[:], AF.Copy)
            pz1 = PS_M.tile([128, G_PADG], F32, name="pz1", tag="pm")
            nc.tensor.matmul(pz1[:], lhsT=w1_b[:, :], rhs=pooledT[:],
                             start=True, stop=True)
            z1 = P.tile([128, G_PADG], BF16, name="z1")
            nc.scalar.activation(z1[:], pz1[:], AF.Relu, bias=b1t[:, 0:1])
            po = PS_M.tile([1, G_PADG], F32, name="po", tag="pm")
            nc.tensor.matmul(po[:], lhsT=w2_b[:, :], rhs=z1[:],
                             start=True, stop=True)
            esb = P.tile([1, G_PADG], F32, name="esb")
            nc.scalar.activation(esb[:], po[:], AF.Exp, bias=b2t[:, 0:1])
            osb = P.tile([1, G_PADG], F32, name="osb")
            nc.scalar.activation(osb[:], esb[:], AF.Ln, bias=1.0)
            nc.sync.dma_start(out=d_out[:, :], in_=osb[:])

    nc.compile()
    return nc


# ----------------------------------------------------------------------------
# entry point
# ----------------------------------------------------------------------------

def make_in_maps(inputs, per_core, meta):
    return _make_in_maps(per_core, meta, **{
        k: inputs[k] for k in ("W_emb", "W_msg", "W_ih", "W_hh", "b_ih",
                               "b_hh", "W1", "b1", "W2", "b2")})


def _make_in_maps(per_core, meta, W_emb, W_msg, W_ih, W_hh, b_ih, b_hh,
                  W1, b1, W2, b2):
    bf = ml_dtypes.bfloat16
    shared = dict(
        iota=np.arange(128, dtype=np.float32).reshape(1, 128),
        ident=np.eye(128, dtype=np.float32).astype(bf),
        wemb=np.asarray(W_emb, np.float32).astype(bf),
        wmsg=np.asarray(W_msg, np.float32).astype(bf),
        wih=np.asarray(W_ih, np.float32).astype(bf),
        whh=np.asarray(W_hh, np.float32).astype(bf),
        bihT=np.ascontiguousarray(
            np.asarray(b_ih, np.float32).reshape(3, H).T),
        bhhT=np.ascontiguousarray(
            np.asarray(b_hh, np.float32).reshape(3, H).T),
        w1=np.asarray(W1, np.float32).astype(bf),
        b1=np.asarray(b1, np.float32).reshape(H, 1),
        w2=np.asarray(W2, np.float32).astype(bf),
        b2=np.asarray(b2, np.float32).reshape(1, 1),
        invc=meta["invc"],
    )
    in_maps = []
    for c in range(N_CORES):
        m = dict(shared)
        m["xT"] = per_core[c]["xT"]
        m["gmat"] = per_core[c]["gmat"]
        for li in range(len(LAYOUTS)):
            m[f"idx{li}"] = per_core[c][f"idx{li}"]
            m[f"slot{li}"] = per_core[c][f"slot{li}"]
        in_maps.append(m)
    return in_maps


def kernel(x, edge_index, batch, W_emb, W_msg, W_ih, W_hh, b_ih, b_hh,
           W1, b1, W2, b2):
    per_core, meta = _preprocess(x, edge_index, batch)
    nc = _build(meta)
    in_maps = _make_in_maps(per_core, meta, W_emb, W_msg, W_ih, W_hh,
                            b_ih, b_hh, W1, b1, W2, b2)

    trace = bool(int(os.environ.get("KERNEL_TRACE", "0")))
    res = run_bass_kernel_spmd(nc, in_maps, list(range(N_CORES)), trace=trace)
    LAST_RESULTS["exec_time_ns"] = res.exec_time_ns
    LAST_RESULTS["profile_json"] = res.profile_json
    LAST_RESULTS["nc"] = nc
    LAST_RESULTS["in_maps"] = in_maps

    return np.asarray(res.results[0]["out"][0, :N_GRAPHS], np.float32)



# revision 88
# speedup vs baseline: 1.0295x; 1.0002x over previous
"""Trainium2 Bass kernel for BondingGraphGNN (gnn_message_passing), v3.

Model (see reference):
  h = relu(x @ W_emb)
  4x: m = h @ W_msg[i]; agg = scatter_add(m[src] -> dst); h = GRU(agg, h)
  h = relu(h); pooled = segment_mean(h, batch); out = softplus(relu(pooled@W1+b1)@W2+b2)

Distribution: even node sharding (6250 nodes/core, padded). Per step each core
computes messages for its nodes, AllGathers the message table to DRAM, and
aggregates its incoming edges locally with a gather + one-hot-matmul
segment-sum, then runs the GRU.

v3 performance structure (~2.3x over v2 in the cost model):
- fp8e4 message table in pair-rows ([2 nodes, 256B] per row, parity-pure edge
  tiles pick their half via the lhsT offset): halves AllGather bytes; the
  scatter-sum averages ~16 messages so fp8 noise washes out (rel err ~1e-3).
- DoubleRow fp8 matmuls: two 128-edge tiles (256-deep contraction) per PE
  instruction via a custom strided lhsT AP - halves PE sequencer work, which
  otherwise bounds the aggregation.
- One DVE is_equal per (group, segment) cell builds the whole one-hot St
  stack (consecutive slot columns, stride-0 broadcast APs).
- Per-step AllGather in 3 pieces [16,17,17] tiles, consumed pass-by-pass
  (pass 0 initializes aggT, later passes accumulate). The small piece 0
  means the next step's first AllGather only waits on 4 GRU chunks, and the
  last pass's gathers hide under its flight - the collective engine streams
  nearly back-to-back.
- Step-invariant gather indices/slots preloaded once; startup reordered so
  the embedding's xT DMA precedes bulk prefetches; shard writes split at
  chunk boundaries so each fires as messages complete; readout folded into
  the last step's chunk loop; pool reduction via AllGather + local sum
  (cheaper than AllReduce); a tiny warmup AllGather absorbs communicator
  bootstrap before the first real collective.

Readout: per-core partial pooling + AllGather + local sum + tiny MLP
replicated on every core (host takes core 0).
"""

import os
import numpy as np

# the trimmed axon package in some containers lacks the NTFF profile hook
# module; stub it so run_bass_kernel_spmd(trace=True) degrades gracefully.
import sys as _sys, types as _types
try:
    import antenv.axon_hooks  # noqa: F401
except Exception:
    _m = _types.ModuleType("antenv.axon_hooks")
    _m.get_axon_ntff_profile_hook = lambda: None
    _sys.modules["antenv.axon_hooks"] = _m

import ml_dtypes
import concourse.bacc as bacc
import concourse.bass as bass
import concourse.mybir as mybir
import concourse.tile as tile
from concourse.bass_utils import run_bass_kernel_spmd

F32 = mybir.dt.float32
BF16 = mybir.dt.bfloat16
F8 = mybir.dt.float8e4
I16 = mybir.dt.int16
AF = mybir.ActivationFunctionType

N_NODES = 50000
N_EDGES = 800000
FEAT = 90
H = 128
STEPS = 4
N_GRAPHS = 100
N_CORES = 8

NC_NODES = N_NODES // N_CORES               # 6250 real nodes per core
N_PAD = 6400
NT = N_PAD // 128                           # dst groups per core (50)
N_CHUNKS = (NT + 3) // 4                    # 13 chunks of <=4 groups
G_PADG = 112                                # padded global graph count
PAD_SLOT = 255.0                            # sentinel slot -> all-zero S row

_TAB_NAME = os.environ.get("K_TAB", "f8")   # f32 | bf16 | f8
ROW_N = 2 if _TAB_NAME == "f8" else 1       # table nodes per row (fp8 pairs)
ROW_W = ROW_N * 128                         # table row width, elements


def _mk_layout(seg_tiles):
    """Per-step AllGather piece layout. Small first piece: the next step's
    first AllGather only waits on a few GRU chunks. The pieces need to cover
    only the REAL sources (0..NC_NODES-1); trailing pad tiles are neither
    messaged nor gathered."""
    assert NC_NODES <= sum(seg_tiles) * 128 <= N_PAD
    nodes = [t * 128 for t in seg_tiles]
    start = [sum(nodes[:q]) for q in range(len(seg_tiles))]
    rows = [N_CORES * n for n in nodes]
    assert max(rows) // ROW_N <= 32768
    return dict(tiles=seg_tiles, nodes=nodes, start=start, rows=rows,
                n_ag=len(seg_tiles))


def _parse_segs(env, default):
    v = os.environ.get(env, "")
    return [int(x) for x in v.split(",")] if v else default


# per-step piece layouts (a distinct last-step layout is supported via
# K_SEGS_LAST but the uniform split benches best)
LAYOUTS = [_mk_layout(_parse_segs("K_SEGS", [16, 17, 16])),
           _mk_layout(_parse_segs("K_SEGS_LAST", [16, 17, 16]))]
STEP_LAYOUT = [0, 0, 0, 1]
if LAYOUTS[1]["tiles"] == LAYOUTS[0]["tiles"]:
    LAYOUTS = LAYOUTS[:1]
    STEP_LAYOUT = [0, 0, 0, 0]

TAB_DT_F32 = _TAB_NAME == "f32"
LAST_RESULTS = {}   # stash for test.py (exec time etc)


def _chunk_groups(ch):
    return range(ch * 4, min((ch + 1) * 4, NT))


# ----------------------------------------------------------------------------
# host-side layout
# ----------------------------------------------------------------------------

def _edge_layout(lay, d_core, grp, slot, s_core, s_local):
    """Tile/slot/idx tables for one AllGather piece layout."""
    n_seg = lay["n_ag"]
    bounds = np.asarray(lay["start"][1:]
                        + [lay["start"][-1] + lay["nodes"][-1]], np.int64)
    seg = np.searchsorted(bounds, s_local, side="right")
    assert seg.max() < n_seg, "pieces must cover all real sources"
    seg_nodes = np.asarray(lay["nodes"], np.int64)[seg]
    seg_start = np.asarray(lay["start"], np.int64)[seg]
    loc = s_core * seg_nodes + (s_local - seg_start)
    if ROW_N == 2:
        # fp8 pair-row table: row r of a piece holds sources (2r, 2r+1);
        # each tile is parity-pure so the matmul lhsT picks one half.
        par = loc % 2
        idxval = loc // 2
    else:
        par = np.zeros_like(loc)
        idxval = loc

    # per (core, grp, seg, par) counts -> uniform budgets
    cnt = np.zeros((N_CORES, NT, n_seg, ROW_N), np.int64)
    np.add.at(cnt, (d_core, grp, seg, par), 1)
    budget = np.ceil(cnt.max(axis=0) / 128).astype(np.int64)
    # every group needs >=1 pass-0 tile so the copy flush writes aggT
    need = budget[:, 0, :].sum(axis=1) == 0
    budget[need, 0, 0] = 1

    # tile order: seg-major, then chunk, then group, then parity
    tb = np.zeros((NT, n_seg, ROW_N), np.int64)
    chunk_t0 = np.zeros((N_CHUNKS, n_seg), np.int64)
    chunk_nt = np.zeros((N_CHUNKS, n_seg), np.int64)
    t = 0
    for s in range(n_seg):
        for ch in range(N_CHUNKS):
            chunk_t0[ch, s] = t
            for g in _chunk_groups(ch):
                for p in range(ROW_N):
                    tb[g, s, p] = t
                    t += int(budget[g, s, p])
            chunk_nt[ch, s] = t - chunk_t0[ch, s]
    t_tot = t

    # edge placement (vectorized)
    order = np.lexsort((par, grp, seg, d_core))
    sc = d_core[order]
    ss = seg[order]
    sg = grp[order]
    sp = par[order]
    sidx = idxval[order]
    sslot = slot[order]
    rid = ((sc * n_seg + ss) * NT + sg) * ROW_N + sp
    run_first = np.r_[0, np.flatnonzero(np.diff(rid)) + 1]
    run_len = np.diff(np.r_[run_first, len(rid)])
    k = np.arange(len(rid)) - np.repeat(run_first, run_len)
    tt = tb[sg, ss, sp] + k // 128
    pp = k % 128
    ct0 = chunk_t0[sg // 4, ss]
    pos = (tt - ct0) * 128 + pp

    idx_arr = np.zeros((N_CORES, 16, t_tot * 8), np.int16)
    slot_arr = np.full((N_CORES, 128, t_tot), PAD_SLOT, np.float32)
    idx_arr[sc, pos % 16, ct0 * 8 + pos // 16] = sidx.astype(np.int16)
    slot_arr[sc, pp, tt] = sslot
    return dict(budget=budget, tb=tb, chunk_t0=chunk_t0, chunk_nt=chunk_nt,
                t_tot=t_tot, idx_arr=idx_arr, slot_arr=slot_arr)


def _preprocess(x, edge_index, batch):
    batch = np.asarray(batch, np.int64)
    src = np.asarray(edge_index[0], np.int64)
    dst = np.asarray(edge_index[1], np.int64)
    frac = float(os.environ.get("K_EDGE_FRAC", "1"))
    if frac < 1.0:  # timing experiments only - wrong results
        n = int(len(src) * frac)
        src, dst = src[:n], dst[:n]

    d_core = dst // NC_NODES
    d_local = dst - d_core * NC_NODES
    grp = d_local // 128
    slot = (d_local % 128).astype(np.float32)
    s_core = src // NC_NODES
    s_local = src - s_core * NC_NODES

    els = [_edge_layout(lay, d_core, grp, slot, s_core, s_local)
           for lay in LAYOUTS]

    # per-core node features (transposed, padded, bf16) and graph one-hots
    counts = np.bincount(batch, minlength=N_GRAPHS).astype(np.float32)
    invc = np.zeros((G_PADG, 1), np.float32)
    invc[:N_GRAPHS, 0] = 1.0 / np.maximum(counts, 1.0)
    x = np.asarray(x, np.float32)
    slot_dt = np.float32 if TAB_DT_F32 else ml_dtypes.bfloat16
    per_core = []
    for c in range(N_CORES):
        n0 = c * NC_NODES
        xT = np.zeros((FEAT, N_PAD), np.float32)
        xT[:, :NC_NODES] = x[n0:n0 + NC_NODES].T
        gmat = np.zeros((128, NT * G_PADG), np.float32)
        l = np.arange(NC_NODES)
        gmat[l % 128, (l // 128) * G_PADG + batch[n0:n0 + NC_NODES]] = 1.0
        pc = dict(
            xT=xT.astype(ml_dtypes.bfloat16),
            gmat=gmat.astype(ml_dtypes.bfloat16),
        )
        for li, el in enumerate(els):
            pc[f"idx{li}"] = np.tile(el["idx_arr"][c], (8, 1))
            pc[f"slot{li}"] = el["slot_arr"][c].astype(slot_dt)
        per_core.append(pc)

    meta = dict(els=els, invc=invc)
    return per_core, meta


# ----------------------------------------------------------------------------
# device program
# ----------------------------------------------------------------------------

def _build(meta):
    DBG_STEPS = int(os.environ.get("K_STEPS", STEPS))
    DBG_NO_AG = bool(int(os.environ.get("K_NO_AG", "0")))
    DBG_NO_AGG = bool(int(os.environ.get("K_NO_AGG", "0")))
    DBG_NO_GRU = bool(int(os.environ.get("K_NO_GRU", "0")))
    els = meta["els"]

    nc = bacc.Bacc("TRN2", target_bir_lowering=False, debug=False,
                   num_devices=N_CORES)

    d_xT = nc.dram_tensor("xT", [FEAT, N_PAD], BF16, kind="ExternalInput")
    SLOT_DT = F32 if TAB_DT_F32 else BF16
    d_idx = [nc.dram_tensor(f"idx{li}", [128, el["t_tot"] * 8], I16,
                            kind="ExternalInput")
             for li, el in enumerate(els)]
    d_slot = [nc.dram_tensor(f"slot{li}", [128, el["t_tot"]], SLOT_DT,
                             kind="ExternalInput")
              for li, el in enumerate(els)]
    d_gmat = nc.dram_tensor("gmat", [128, NT * G_PADG], BF16,
                            kind="ExternalInput")
    d_invc = nc.dram_tensor("invc", [G_PADG, 1], F32, kind="ExternalInput")
    d_iota = nc.dram_tensor("iota", [1, 128], F32, kind="ExternalInput")
    d_ident = nc.dram_tensor("ident", [128, 128], BF16, kind="ExternalInput")
    d_wemb = nc.dram_tensor("wemb", [FEAT, H], BF16, kind="ExternalInput")
    d_wmsg = nc.dram_tensor("wmsg", [STEPS, H, H], BF16, kind="ExternalInput")
    d_wih = nc.dram_tensor("wih", [H, 3 * H], BF16, kind="ExternalInput")
    d_whh = nc.dram_tensor("whh", [H, 3 * H], BF16, kind="ExternalInput")
    d_bihT = nc.dram_tensor("bihT", [H, 3], F32, kind="ExternalInput")
    d_bhhT = nc.dram_tensor("bhhT", [H, 3], F32, kind="ExternalInput")
    d_w1 = nc.dram_tensor("w1", [H, H], BF16, kind="ExternalInput")
    d_b1 = nc.dram_tensor("b1", [H, 1], F32, kind="ExternalInput")
    d_w2 = nc.dram_tensor("w2", [H, 1], BF16, kind="ExternalInput")
    d_b2 = nc.dram_tensor("b2", [1, 1], F32, kind="ExternalInput")
    d_out = nc.dram_tensor("out", [1, G_PADG], F32, kind="ExternalOutput")

    with tile.TileContext(nc) as tc:
        with (
            tc.tile_pool(name="persist", bufs=1) as P,
            tc.tile_pool(name="dram", bufs=1, space="DRAM") as DR,
            tc.tile_pool(name="epool",
                         bufs=int(os.environ.get("K_EP", "4"))) as EP,
            tc.tile_pool(name="spool",
                         bufs=int(os.environ.get("K_SPOOL", "8"))) as SP,
            tc.tile_pool(name="gpool",
                         bufs=int(os.environ.get("K_GP", "3"))) as GP,
            tc.tile_pool(name="ps_agg", space="PSUM",
                         bufs=int(os.environ.get("K_PA", "2"))) as PS_AGG,
            tc.tile_pool(name="ps_m", bufs=1, space="PSUM") as PS_M,
            tc.tile_pool(name="ps_gru", space="PSUM",
                         bufs=int(os.environ.get("K_PG", "4"))) as PS_GRU,
        ):
            # DRAM temps: per-step message shards and gathered tables
            TDT = {"f32": F32, "bf16": BF16, "f8": F8}[_TAB_NAME]
            n_tab = max(DBG_STEPS, 1)

            def lay_of(step):
                return LAYOUTS[STEP_LAYOUT[min(step, STEPS - 1)]]

            shard_bufs = [[DR.tile([lay_of(st)["nodes"][q] // ROW_N, ROW_W],
                                   TDT, name=f"m_shard{st}_{q}")
                           for q in range(lay_of(st)["n_ag"])]
                          for st in range(n_tab)]
            table_bufs = [[DR.tile([lay_of(st)["rows"][q] // ROW_N, ROW_W],
                                   TDT, addr_space="Shared",
                                   name=f"m_table{st}_{q}")
                           for q in range(lay_of(st)["n_ag"])]
                          for st in range(n_tab)]

            # ------- startup-critical loads first (xT feeds embedding) -----
            def load(dram_ap, shape, name, dt=BF16):
                tl = P.tile(shape, dt, name=name)
                nc.sync.dma_start(out=tl[:], in_=dram_ap)
                return tl

            xT_b = P.tile([FEAT, N_PAD], BF16, name="xT_b")
            nc.sync.dma_start(out=xT_b[:], in_=d_xT[:, :])
            wemb_b = load(d_wemb[:, :], [FEAT, H], "wemb")
            wmsg_b = [load(d_wmsg[s, :, :], [H, H], f"wmsg{s}")
                      for s in range(STEPS)]
            iota_f = P.tile([128, 128], F32, name="iota_f")
            nc.sync.dma_start(out=iota_f[:],
                              in_=d_iota.ap().to_broadcast([128, 128]))
            iota_b = P.tile([128, 128], BF16, name="iota_b")
            nc.vector.tensor_copy(iota_b[:], iota_f[:])
            ident_b = P.tile([128, 128], BF16, name="ident_b")
            nc.sync.dma_start(out=ident_b[:], in_=d_ident[:, :])

            # state
            h_t = P.tile([128, N_PAD], BF16, name="h_t")
            m_all = P.tile([128, N_PAD], TDT, name="m_all")
            aggT = P.tile([128, N_PAD], BF16, name="aggT")

            # warmup barrier: a tiny collective with no data deps issues at
            # kernel start, absorbing communicator bootstrap cost/skew before
            # the first real AllGather.
            d_warm_in = DR.tile([1, 2], F32, name="warm_in")
            d_warm_out = DR.tile([8, 2], F32, addr_space="Shared",
                                 name="warm_out")
            warm_t = P.tile([1, 2], F32, name="warm_t")
            nc.vector.memset(warm_t[:], 0.0)
            nc.sync.dma_start(out=d_warm_in[:, :], in_=warm_t[:])
            nc.gpsimd.collective_compute(
                "AllGather", mybir.AluOpType.bypass,
                ins=[d_warm_in.opt()], outs=[d_warm_out.opt()],
                replica_groups=[list(range(N_CORES))],
            )

            def msg_tile(t, step):
                pm = PS_M.tile([128, 128], F32, name="pm", tag="pm")
                nc.tensor.matmul(pm[:], lhsT=h_t[:, t * 128:(t + 1) * 128],
                                 rhs=wmsg_b[step % STEPS][:, :],
                                 start=True, stop=True)
                nc.scalar.activation(m_all[:, t * 128:(t + 1) * 128],
                                     pm[:], AF.Copy)

            def send_seg(step, q):
                """DMA m_all segment q to its shard and AllGather it.

                The shard write is split at 512-node chunk boundaries so each
                sub-DMA fires as soon as its chunk's messages are done - the
                collective then only waits on the last small piece."""
                lay = lay_of(step)
                shard = shard_bufs[step][q]
                n0, nn = lay["start"][q], lay["nodes"][q]
                cuts = [n0] + [b for b in range((n0 // 512 + 1) * 512,
                                                n0 + nn, 512)] + [n0 + nn]
                for lo, hi in zip(cuts[:-1], cuts[1:]):
                    src = m_all[:, lo:hi]
                    if ROW_N == 2:
                        # pair-row layout: row r = nodes (2r, 2r+1); node
                        # n=a*128+p lands at row a*64+p//2, offset (p%2)*128
                        out_ap = shard[(lo - n0) // 2:(hi - n0) // 2,
                                       :].rearrange(
                            "(a i) (e b) -> (i e) a b", i=64, e=2)
                    else:
                        out_ap = shard[lo - n0:hi - n0, :].rearrange(
                            "(a p) b -> p a b", p=128)
                    nc.sync.dma_start(
                        out=out_ap,
                        in_=src.rearrange("p (a b) -> p a b", b=128))
                if not DBG_NO_AG:
                    nc.gpsimd.collective_compute(
                        "AllGather", mybir.AluOpType.bypass,
                        ins=[shard.opt()],
                        outs=[table_bufs[step][q].opt()],
                        replica_groups=[list(range(N_CORES))],
                    )

            def ag_after_chunk_for(step):
                """chunk idx after which msg tiles for AG piece q are done"""
                lay = lay_of(step)
                m = {}
                for q in range(lay["n_ag"]):
                    last_tile = (lay["start"][q] + lay["nodes"][q]) // 128 - 1
                    m.setdefault(last_tile // 4, []).append(q)
                return m

            def gru_chunk(ch, step):
                off = ch * 512
                size = min(512, N_PAD - off)
                sl = slice(off, off + size)
                p_r = PS_GRU.tile([128, size], F32, name="p_r", tag="pgru")
                nc.tensor.matmul(p_r[:], lhsT=wih_b[:, 0:128],
                                 rhs=aggT[:, sl], start=True, stop=False)
                nc.tensor.matmul(p_r[:], lhsT=whh_b[:, 0:128],
                                 rhs=h_t[:, sl], start=False, stop=True)
                p_z = PS_GRU.tile([128, size], F32, name="p_z", tag="pgru")
                nc.tensor.matmul(p_z[:], lhsT=wih_b[:, 128:256],
                                 rhs=aggT[:, sl], start=True, stop=False)
                nc.tensor.matmul(p_z[:], lhsT=whh_b[:, 128:256],
                                 rhs=h_t[:, sl], start=False, stop=True)
                p_xn = PS_GRU.tile([128, size], F32, name="p_xn", tag="pgru")
                nc.tensor.matmul(p_xn[:], lhsT=wih_b[:, 256:384],
                                 rhs=aggT[:, sl], start=True, stop=True)
                p_hn = PS_GRU.tile([128, size], F32, name="p_hn", tag="pgru")
                nc.tensor.matmul(p_hn[:], lhsT=whh_b[:, 256:384],
                                 rhs=h_t[:, sl], start=True, stop=True)
                r_t = GP.tile([128, size], BF16, name="r_t", tag="gp1")
                nc.scalar.activation(r_t[:], p_r[:], AF.Sigmoid,
                                     bias=bsum[:, 0:1])
                z_t = GP.tile([128, size], BF16, name="z_t", tag="gp2")
                nc.scalar.activation(z_t[:], p_z[:], AF.Sigmoid,
                                     bias=bsum[:, 1:2])
                hn_t = GP.tile([128, size], BF16, name="hn_t", tag="gp3")
                nc.scalar.activation(hn_t[:], p_hn[:], AF.Identity,
                                     bias=bhh[:, 2:3])
                t1 = GP.tile([128, size], BF16, name="t1", tag="gp4")
                nc.vector.tensor_mul(t1[:], r_t[:], hn_t[:])
                u_t = GP.tile([128, size], F32, name="u_t", tag="gp5")
                nc.vector.tensor_add(u_t[:], t1[:], p_xn[:])
                n_t = GP.tile([128, size], F32, name="n_t", tag="gp6")
                nc.scalar.activation(n_t[:], u_t[:], AF.Tanh,
                                     bias=bih[:, 2:3])
                d_t = GP.tile([128, size], F32, name="d_t", tag="gp7")
                nc.vector.tensor_sub(d_t[:], h_t[:, sl], n_t[:])
                e_t = GP.tile([128, size], F32, name="e_t", tag="gp8")
                nc.vector.tensor_mul(e_t[:], z_t[:], d_t[:])
                nc.vector.tensor_add(h_t[:, sl], n_t[:], e_t[:])

            pool_acc = P.tile([G_PADG, 128], F32, name="pool_acc")

            def readout_chunk(ch):
                # relu commutes with the transpose, so the PSUM->SBUF copy
                # after the transpose applies it - no separate relu pass
                ts = list(range(ch * 4, min((ch + 1) * 4, NT)))
                pq = PS_M.tile([G_PADG, 128], F32, name="pq", tag="pq",
                               bufs=1)
                for i, t in enumerate(ts):
                    ptr2 = PS_M.tile([128, 128], BF16, name="ptr2", tag="pm")
                    nc.tensor.transpose(ptr2[:],
                                        h_t[:, t * 128:(t + 1) * 128],
                                        ident_b[:])
                    hnm = GP.tile([128, 128], BF16, name="hnm", tag="gp1")
                    nc.scalar.activation(hnm[:], ptr2[:], AF.Relu)
                    nc.tensor.matmul(
                        pq[:], lhsT=gmat_b[:, t * G_PADG:(t + 1) * G_PADG],
                        rhs=hnm[:], start=(i == 0), stop=(i == len(ts) - 1))
                if ch == 0:
                    nc.scalar.activation(pool_acc[:], pq[:], AF.Copy)
                else:
                    nc.vector.tensor_add(pool_acc[:], pool_acc[:], pq[:])

            def embed_chunk(ch):
                off = ch * 512
                size = min(512, N_PAD - off)
                pe = PS_GRU.tile([128, size], F32, name="pe_emb", tag="pgru")
                nc.tensor.matmul(pe[:], lhsT=wemb_b[:, :],
                                 rhs=xT_b[:, off:off + size],
                                 start=True, stop=True)
                nc.scalar.activation(h_t[:, off:off + size], pe[:], AF.Relu)

            # deferred loads: needed only after the first AllGather lands
            slot_tiles, ix_tiles = [], []
            for li, el in enumerate(els):
                st_t = P.tile([128, el["t_tot"]], SLOT_DT, name=f"slot_a{li}")
                nc.sync.dma_start(out=st_t[:], in_=d_slot[li][:, :])
                slot_tiles.append(st_t)
                ix_t = P.tile([128, el["t_tot"] * 8], I16, name=f"ix_a{li}")
                nc.sync.dma_start(out=ix_t[:], in_=d_idx[li][:, :])
                ix_tiles.append(ix_t)
            wih_b = load(d_wih[:, :], [H, 3 * H], "wih")
            whh_b = load(d_whh[:, :], [H, 3 * H], "whh")
            bih = load(d_bihT[:, :], [H, 3], "bih", F32)
            bhh = load(d_bhhT[:, :], [H, 3], "bhh", F32)
            bsum = P.tile([H, 3], F32, name="bsum")
            nc.vector.tensor_add(bsum[:], bih[:], bhh[:])
            w1_b = load(d_w1[:, :], [H, H], "w1")
            w2_b = load(d_w2[:, :], [H, 1], "w2")
            b1t = load(d_b1[:, :], [H, 1], "b1t", F32)
            b2t = load(d_b2[:, :], [1, 1], "b2t", F32)
            invc_t = load(d_invc[:, :], [G_PADG, 1], "invc_t", F32)
            gmat_b = load(d_gmat[:, :], [128, NT * G_PADG], "gmat")

            # ---------------- message-passing steps ----------------
            if DBG_STEPS > 0:
                lay0 = lay_of(0)
                # embed + message piece-by-piece so AllGather piece q is not
                # queued behind later chunks' embedding work
                emb_done = 0
                for q in range(lay0["n_ag"]):
                    t0q = lay0["start"][q] // 128
                    need_ch = min((t0q + lay0["tiles"][q] + 3) // 4, N_CHUNKS)
                    while emb_done < need_ch:
                        embed_chunk(emb_done)
                        emb_done += 1
                    for t in range(t0q, t0q + lay0["tiles"][q]):
                        msg_tile(t, 0)
                    send_seg(0, q)
                while emb_done < N_CHUNKS:
                    embed_chunk(emb_done)
                    emb_done += 1
            else:
                for ch in range(N_CHUNKS):
                    embed_chunk(ch)

            for step in range(DBG_STEPS):
                lay = lay_of(step)
                el = els[STEP_LAYOUT[min(step, STEPS - 1)]]
                budget, tb = el["budget"], el["tb"]
                chunk_t0, chunk_nt = el["chunk_t0"], el["chunk_nt"]
                li = STEP_LAYOUT[min(step, STEPS - 1)]
                ix_all, slot_all = ix_tiles[li], slot_tiles[li]
                tab_half = [table_bufs[step][q][:, :]
                            for q in range(lay["n_ag"])]
                ag_after_chunk = (ag_after_chunk_for(step + 1)
                                  if step + 1 < DBG_STEPS else {})

                if DBG_NO_AGG:
                    nc.vector.memset(aggT[:], 0.0)
                for s in range(lay["n_ag"]):
                    last = s == lay["n_ag"] - 1
                    for ch in range(N_CHUNKS):
                        if not DBG_NO_AGG:
                            t0 = int(chunk_t0[ch, s])
                            n_ch = int(chunk_nt[ch, s])
                            if n_ch > 0:
                                E = EP.tile([128, n_ch, ROW_W], TDT,
                                            name="E", tag="E")
                                nc.gpsimd.dma_gather(
                                    E[:], tab_half[s],
                                    ix_all[:, t0 * 8:(t0 + n_ch) * 8],
                                    n_ch * 128, n_ch * 128, ROW_W,
                                    single_packet=bool(int(
                                        os.environ.get("K_SP", "0"))))
                            for g in _chunk_groups(ch):
                                tl = [(int(tb[g, s, p]) + kk, p)
                                      for p in range(ROW_N)
                                      for kk in range(int(budget[g, s, p]))]
                                if not tl:
                                    continue
                                pa = PS_AGG.tile([128, 128], F32, name="pa",
                                                 tag="pa")
                                # one is_equal builds the one-hot St for the
                                # whole cell (tiles are consecutive, so slot
                                # columns broadcast with stride-0 inner dim)
                                nb = len(tl)
                                tg0 = tl[0][0]
                                St_c = SP.tile([128, nb, 128], TDT,
                                               name="St", tag="St")
                                io = (iota_f if TAB_DT_F32 else iota_b)[:]
                                i_rep = bass.AP(
                                    io.tensor, io.offset,
                                    [tuple(io.ap[0]), (0, nb),
                                     tuple(io.ap[1])])
                                sl0 = slot_all[:, tg0:tg0 + 1]
                                s_rep = bass.AP(
                                    sl0.tensor, sl0.offset,
                                    [tuple(sl0.ap[0]), (1, nb), (0, 128)])
                                nc.vector.tensor_tensor(
                                    St_c[:], i_rep, s_rep,
                                    mybir.AluOpType.is_equal)
                                # DoubleRow fp8: two edge tiles per matmul
                                # (256-deep contraction) to halve PE SEQ work
                                if ROW_N == 2:
                                    pairs = [tl[j:j + 2]
                                             for j in range(0, len(tl), 2)]
                                else:
                                    pairs = [tl[j:j + 1]
                                             for j in range(len(tl))]
                                for j, pr in enumerate(pairs):
                                    st_fl = (j == 0, j == len(pairs) - 1)
                                    if len(pr) == 2:
                                        (ta, pa_), (tb_, pb_) = pr
                                        a0 = E[:, ta - t0,
                                               pa_ * 128:pa_ * 128 + 128]
                                        delta = ((tb_ - ta) * ROW_W
                                                 + (pb_ - pa_) * 128)
                                        lhsT2 = bass.AP(
                                            a0.tensor, a0.offset,
                                            [tuple(a0.ap[0]), (delta, 2),
                                             tuple(a0.ap[1])])
                                        nc.tensor.matmul(
                                            pa[:], lhsT=lhsT2,
                                            rhs=St_c[:, 2 * j:2 * j + 2, :],
                                            perf_mode=(mybir.MatmulPerfMode
                                                       .DoubleRow),
                                            start=st_fl[0], stop=st_fl[1])
                                    else:
                                        tg, p = pr[0]
                                        nc.tensor.matmul(
                                            pa[:],
                                            lhsT=E[:, tg - t0,
                                                   p * 128:(p + 1) * 128],
                                            rhs=St_c[:, tg - tg0, :],
                                            start=st_fl[0],
                                            stop=st_fl[1])
                                sl = slice(g * 128, (g + 1) * 128)
                                if s == 0:
                                    nc.scalar.activation(aggT[:, sl], pa[:],
                                                         AF.Copy)
                                else:
                                    nc.vector.tensor_add(aggT[:, sl],
                                                         aggT[:, sl], pa[:])
                        if last:
                            if not DBG_NO_GRU:
                                gru_chunk(ch, step)
                            if step + 1 < DBG_STEPS:
                                lay_n = lay_of(step + 1)
                                t_hi = (lay_n["start"][-1]
                                        + lay_n["nodes"][-1]) // 128
                                for t in range(ch * 4,
                                               min((ch + 1) * 4, t_hi)):
                                    msg_tile(t, step + 1)
                                for q in ag_after_chunk.get(ch, []):
                                    send_seg(step + 1, q)
                            else:
                                # final step: fold the readout (relu +
                                # transpose + pool matmul) into the chunk
                                # loop so the tail doesn't serialize
                                readout_chunk(ch)

            # ---------------- readout ----------------
            if DBG_STEPS == 0:
                for ch in range(N_CHUNKS):
                    readout_chunk(ch)
            # cross-core pool reduction: AllGather + local sum is cheaper
            # than AllReduce (no 1.875x collective penalty); bf16 partials
            # halve the collective bytes (pooled means tolerate the rounding)
            pool_bf = P.tile([G_PADG, 128], BF16, name="pool_bf")
            nc.vector.tensor_copy(pool_bf[:], pool_acc[:])
            d_pool_in = DR.tile([G_PADG, H], BF16, name="pool_in")
            d_pool_out = DR.tile([N_CORES * G_PADG, H], BF16,
                                 addr_space="Shared", name="pool_out")
            nc.sync.dma_start(out=d_pool_in[:, :], in_=pool_bf[:])
            nc.gpsimd.collective_compute(
                "AllGather", mybir.AluOpType.bypass,
                ins=[d_pool_in.opt()], outs=[d_pool_out.opt()],
                replica_groups=[list(range(N_CORES))],
            )
            pr8 = P.tile([G_PADG, N_CORES, 128], BF16, name="pr8")
            nc.sync.dma_start(
                out=pr8[:],
                in_=d_pool_out.rearrange("(r g) f -> g r f", g=G_PADG))
            pool_r = P.tile([G_PADG, 128], F32, name="pool_r")
            nc.vector.tensor_add(pool_r[:], pr8[:, 0, :], pr8[:, 1, :])
            for r in range(2, N_CORES):
                nc.vector.tensor_add(pool_r[:], pool_r[:], pr8[:, r, :])
            pooled = P.tile([G_PADG, 128], BF16, name="pooled")
            nc.vector.tensor_scalar(pooled[:], pool_r[:], invc_t[:], None,
                                    mybir.AluOpType.mult)
            ppt = PS_M.tile([128, G_PADG], BF16, name="ppt", tag="pm")
            nc.tensor.transpose(ppt[:], pooled[:],
                                ident_b[0:G_PADG, 0:G_PADG])
            pooledT = P.tile([128, G_PADG], BF16, name="pooledT")
            nc.scalar.activation(pooledT[:], ppt[:], AF.Copy)
            pz1 = PS_M.tile([128, G_PADG], F32, name="pz1", tag="pm")
            nc.tensor.matmul(pz1[:], lhsT=w1_b[:, :], rhs=pooledT[:],
                             start=True, stop=True)
            z1 = P.tile([128, G_PADG], BF16, name="z1")
            nc.scalar.activation(z1[:], pz1[:], AF.Relu, bias=b1t[:, 0:1])
            po = PS_M.tile([1, G_PADG], F32, name="po", tag="pm")
            nc.tensor.matmul(po[:], lhsT=w2_b[:, :], rhs=z1[:],
                             start=True, stop=True)
            esb = P.tile([1, G_PADG], F32, name="esb")
            nc.scalar.activation(esb[:], po[:], AF.Exp, bias=b2t[:, 0:1])
            osb = P.tile([1, G_PADG], F32, name="osb")
            nc.scalar.activation(osb[:], esb[:], AF.Ln, bias=1.0)
            nc.sync.dma_start(out=d_out[:, :], in_=osb[:])

    nc.compile()
    return nc


# ----------------------------------------------------------------------------
# entry point
# ----------------------------------------------------------------------------

def make_in_maps(inputs, per_core, meta):
    return _make_in_maps(per_core, meta, **{
        k: inputs[k] for k in ("W_emb", "W_msg", "W_ih", "W_hh", "b_ih",
                               "b_hh", "W1", "b1", "W2", "b2")})


def _make_in_maps(per_core, meta, W_emb, W_msg, W_ih, W_hh, b_ih, b_hh,
                  W1, b1, W2, b2):
    bf = ml_dtypes.bfloat16
    shared = dict(
        iota=np.arange(128, dtype=np.float32).reshape(1, 128),
        ident=np.eye(128, dtype=np.float32).astype(bf),
        wemb=np.asarray(W_emb, np.float32).astype(bf),
        wmsg=np.asarray(W_msg, np.float32).astype(bf),
        wih=np.asarray(W_ih, np.float32).astype(bf),
        whh=np.asarray(W_hh, np.float32).astype(bf),
        bihT=np.ascontiguousarray(
            np.asarray(b_ih, np.float32).reshape(3, H).T),
        bhhT=np.ascontiguousarray(
            np.asarray(b_hh, np.float32).reshape(3, H).T),
        w1=np.asarray(W1, np.float32).astype(bf),
        b1=np.asarray(b1, np.float32).reshape(H, 1),
        w2=np.asarray(W2, np.float32).astype(bf),
        b2=np.asarray(b2, np.float32).reshape(1, 1),
        invc=meta["invc"],
    )
    in_maps = []
    for c in range(N_CORES):
        m = dict(shared)
        m["xT"] = per_core[c]["xT"]
        m["gmat"] = per_core[c]["gmat"]
        for li in range(len(LAYOUTS)):
            m[f"idx{li}"] = per_core[c][f"idx{li}"]
            m[f"slot{li}"] = per_core[c][f"slot{li}"]
        in_maps.append(m)
    return in_maps


def kernel(x, edge_index, batch, W_emb, W_msg, W_ih, W_hh, b_ih, b_hh,
           W1, b1, W2, b2):
    per_core, meta = _preprocess(x, edge_index, batch)
    nc = _build(meta)
    in_maps = _make_in_maps(per_core, meta, W_emb, W_msg, W_ih, W_hh,
                            b_ih, b_hh, W1, b1, W2, b2)

    trace = bool(int(os.environ.get("KERNEL_TRACE", "0")))
    res = run_bass_kernel_spmd(nc, in_maps, list(range(N_CORES)), trace=trace)
    LAST_RESULTS["exec_time_ns"] = res.exec_time_ns
    LAST_RESULTS["profile_json"] = res.profile_json
    LAST_RESULTS["nc"] = nc
    LAST_RESULTS["in_maps"] = in_maps

    return np.asarray(res.results[0]["out"][0, :N_GRAPHS], np.float32)

